# revision 6
# baseline (speedup 1.0000x reference)
"""Trainium2 Bass kernel for nn_DeformBottleneck (DCNv2 bottleneck block), v2.

Same sharding as v1: 8 shards = (batch b, row-half) on 8 cores; each core
computes output rows [r0, r0+64) of one image.

v2 restructure vs v1:
  - Y-field matmuls write fp32 PSUM in small chunks; evacuation split
    across DVE and ACT with a deep chunk ring.
  - The 81 hat-stencil weight terms per row are TS-mults at 4x on DVE
    (per-row [x,ch] tiles, per-partition scalar), with overflow terms on
    ACT (activation scale) and POOL (broadcast tensor_tensor blocks).
  - Accumulation goes to the Tensor engine: weighted tmp tiles transpose-
    accumulate (matmul vs identity) into a channel-major fp32 PSUM master
    [ch, 16, W] per block; leftover terms accumulate into pixel-major
    bf16 masters on DVE/POOL, folded into the PSUM master at block end.
  - Channel-major master kills the conv3 transposes: bn2+relu / conv3 /
    bn3 / downsample / final relu all run channel-major.
"""

import os
import sys
from contextlib import ExitStack

import numpy as np

sys.path.insert(0, "/opt/trn_rl_repo")

import ml_dtypes

import concourse.bass as bass
from concourse import bacc
import concourse.mybir as mybir
import concourse.tile as tile
from concourse.bass_utils import run_bass_kernel_spmd

BF = ml_dtypes.bfloat16
F32 = mybir.dt.float32
BF16 = mybir.dt.bfloat16
I32 = mybir.dt.int32
AF = mybir.ActivationFunctionType
OP = mybir.AluOpType

B, CIN, H, W = 4, 256, 128, 128
PL, KK = 128, 9
PW = 132          # padded out1 slab width
ROWS_OUT = 64     # output rows per core
MARG = 3
NR1 = ROWS_OUT + 2 * MARG
RB = 16           # rows per block
NBLK = ROWS_OUT // RB
NYR = RB + 2
N_CORES = 8

# chunking of the 18-row Y field into PSUM pieces
YCH = (4, 4, 4, 4, 2)


def _sched(weights, n):
    """Interleaved largest-remainder schedule: n picks from weighted set."""
    tot = float(sum(weights.values()))
    acc = {e: 0.0 for e in weights}
    out = []
    for _ in range(n):
        for e in acc:
            acc[e] += weights[e] / tot
        pick = max(acc, key=lambda e: (acc[e], e))
        acc[pick] -= 1.0
        out.append(pick)
    return out


def _parse_w(s, default):
    # "D52A10P19" -> {'D':52,'A':10,'P':19}
    if not s:
        return default
    out = {}
    key = None
    num = ''
    for ch in s:
        if ch.isalpha():
            if key is not None:
                out[key] = int(num)
            key = ch
            num = ''
        else:
            num += ch
    out[key] = int(num)
    return out


def _build(nc):
    MULT_W = _parse_w(os.environ.get('K2_MULT', ''), {'D': 55, 'A': 8, 'P': 18})
    ACC_W = _parse_w(os.environ.get('K2_ACC', ''), {'E': 66, 'D': 8, 'P': 7})
    # per-term engine assignment, term order: (k-major, dx, dy)
    MULT = _sched(MULT_W, 81)
    ACC = _sched(ACC_W, 81)
    EVAC = _sched(_parse_w(os.environ.get('K2_EVAC', ''), {'D': 1, 'A': 4}), 60)

    def di(name, shape, dt=F32):
        return nc.dram_tensor(name, shape, dt, kind="ExternalInput")

    xs = di("xs", [2, 128, NR1 * W], BF16)
    w1f = di("w1f", [128, 2, 128], BF16)
    t1a = di("t1a", [128, 1])
    s1b = di("s1b", [128, 1])
    t1b = di("t1b", [128, 1])
    woffT = di("woffT", [128, KK, 27], BF16)
    b_off = di("b_off", [27, 1])
    wk = di("wk", [128, KK, 128], BF16)
    iden = di("iden", [128, 128], BF16)
    bdc2c = di("bdc2c", [128, 1])
    w3f = di("w3f", [128, 128], BF16)
    t3a = di("t3a", [128, 1])
    s3b = di("s3b", [128, 1])
    tfin = di("tfin", [128, 1])
    wdsf = di("wdsf", [128, 2, 128], BF16)
    out_d = nc.dram_tensor("out", [128, ROWS_OUT * W], F32, kind="ExternalOutput")

    with tile.TileContext(nc) as tc, ExitStack() as ctx:
        P = lambda name, bufs=1, **kw: ctx.enter_context(
            tc.tile_pool(name=name, bufs=bufs, **kw))
        consts = P("consts")
        big = P("big")
        wts = P("wts")
        work = P("work", bufs=2)

        c_w1 = consts.tile([128, 2, 128], BF16); nc.sync.dma_start(c_w1[:], w1f[:])
        c_t1a = consts.tile([128, 1], F32); nc.sync.dma_start(c_t1a[:], t1a[:])
        c_s1b = consts.tile([128, 1], F32); nc.sync.dma_start(c_s1b[:], s1b[:])
        c_t1b = consts.tile([128, 1], F32); nc.sync.dma_start(c_t1b[:], t1b[:])
        c_woff = consts.tile([128, KK, 27], BF16); nc.sync.dma_start(c_woff[:], woffT[:])
        c_boff = consts.tile([27, 1], F32); nc.sync.dma_start(c_boff[:], b_off[:])
        c_wk = consts.tile([128, KK, 128], BF16); nc.sync.dma_start(c_wk[:], wk[:])
        c_id = consts.tile([128, 128], BF16); nc.sync.dma_start(c_id[:], iden[:])
        c_zero = consts.tile([128, 128], BF16); nc.vector.memset(c_zero[:], 0.0)
        c_bdc2 = consts.tile([128, 1], F32); nc.sync.dma_start(c_bdc2[:], bdc2c[:])
        c_w3 = consts.tile([128, 128], BF16); nc.sync.dma_start(c_w3[:], w3f[:])
        c_t3a = consts.tile([128, 1], F32); nc.sync.dma_start(c_t3a[:], t3a[:])
        c_s3b = consts.tile([128, 1], F32); nc.sync.dma_start(c_s3b[:], s3b[:])
        c_tfin = consts.tile([128, 1], F32); nc.sync.dma_start(c_tfin[:], tfin[:])
        c_wds = consts.tile([128, 2, 128], BF16); nc.sync.dma_start(c_wds[:], wdsf[:])

        xsb0t = big.tile([128, NR1 * W], BF16)
        xsb1t = big.tile([128, NR1 * W], BF16)
        nc.sync.dma_start(xsb0t[:], xs[0])
        nc.sync.dma_start(xsb1t[:], xs[1])

        out1 = big.tile([128, NR1, PW], BF16)
        nc.gpsimd.memset(out1[:], 0.0)

        pre_ctx = tc.tile_pool(name="ps_pre", bufs=2, space="PSUM")
        ps_a = pre_ctx.__enter__()

        # ---- conv1 (1x1 256->128) + BN + relu, twice-relu'd -> out1 slab
        def conv1_iter(it, pool=None, ptag="c1"):
            px0 = it * 2 * W
            pt = (pool or ps_a).tile([128, 2, 128], F32, tag=ptag)
            nc.tensor.matmul(pt[:], c_w1[:, 0, :], xsb0t[:, px0:px0 + 256],
                             start=True, stop=False)
            nc.tensor.matmul(pt[:], c_w1[:, 1, :], xsb1t[:, px0:px0 + 256],
                             start=False, stop=True)
            t = work.tile([128, 2, 128], F32, tag="c1s")
            nc.scalar.activation(t[:], pt[:], AF.Relu, bias=c_t1a[:, :], scale=1.0)
            nc.vector.tensor_scalar(t[:], t[:], c_s1b[:, :], c_t1b[:, :],
                                    op0=OP.mult, op1=OP.add)
            nc.vector.tensor_scalar_max(out1[:, it * 2:it * 2 + 2, 2:130], t[:], 0.0)

        for it in range(11):
            conv1_iter(it)

        pre_ctx.__exit__(None, None, None)
        offp = ctx.enter_context(tc.tile_pool(name="offp", bufs=2))

        ps_y = ctx.enter_context(tc.tile_pool(name="ps_y", bufs=int(os.environ.get("K2_PSYB","3")), space="PSUM"))
        ps_m = ctx.enter_context(tc.tile_pool(name="ps_m", bufs=1, space="PSUM"))
        ps_f = ctx.enter_context(tc.tile_pool(name="ps_f", bufs=1, space="PSUM"))
        yp = ctx.enter_context(tc.tile_pool(name="yp", bufs=int(os.environ.get("K2_YB","5"))))
        tmpp = ctx.enter_context(tc.tile_pool(name="tmpp", bufs=int(os.environ.get("K2_TB","24"))))
        tmpb = ctx.enter_context(tc.tile_pool(name="tmpb", bufs=int(os.environ.get("K2_TBB","4"))))
        fin = ctx.enter_context(tc.tile_pool(name="fin", bufs=2))

        # which accumulate-engines appear (for stop-flag placement)
        has_aD = 'D' in ACC
        has_aP = 'P' in ACC
        last_e_ti = max((ti for ti in range(81) if ACC[ti] == 'E'), default=-1)

        # ---- main loop: per block, stream (k,dx) Y-fields, weight + accum
        units = [(k, dx) for k in range(9) for dx in (-1, 0, 1)]
        evac_i = 0
        def emit_offsets(blk):
            # offsets for block blk: conv, transpose, hat weights g[a][b]
            r0b = blk * RB
            off = offp.tile([27, RB * W], BF16, tag="off")
            for it in range(RB // 4):
                r0 = r0b + it * 4
                pt = ps_f.tile([27, 512], F32, tag="pf")
                for k in range(KK):
                    ky, kx = k // 3, k % 3
                    src = out1[:, r0 + ky + 2:r0 + ky + 6, 1 + kx:1 + kx + W]
                    nc.tensor.matmul(pt[:], c_woff[:, k, :], src,
                                     start=(k == 0), stop=(k == KK - 1))
                nc.scalar.activation(off[:, it * 4 * W:(it + 1) * 4 * W], pt[:],
                                     AF.Identity, bias=c_boff[:, :], scale=1.0)
            offT = offp.tile([128, RB, 28], F32, tag="offT")
            pt = ps_f.tile([128, RB, 28], BF16, tag="pf")
            for j in range(RB):
                nc.tensor.transpose(pt[:, j, 0:27], off[:, j * W:(j + 1) * W],
                                    c_id[0:27, 0:27])
            nc.vector.tensor_copy(offT[:, :, 0:27], pt[:, :, 0:27])

            o1v, o2v, o3v = (offT[:, :, 0:9], offT[:, :, 9:18],
                             offT[:, :, 18:27])
            mask = offp.tile([128, RB, 9], BF16, tag="mask")
            nc.scalar.activation(mask[:], o3v, AF.Sigmoid)
            ay = [offp.tile([128, RB, 9], BF16, name="ayt%d" % i,
                            tag="ayt" + str(i)) for i in range(3)]
            bx = [offp.tile([128, RB, 9], BF16, name="bxt%d" % i,
                            tag="bxt" + str(i)) for i in range(3)]
            tmp = offp.tile([128, RB, 9], BF16, tag="tmp9")
            for (lo, hi, mid, srcv) in ((ay[0], ay[2], ay[1], o1v),
                                        (bx[0], bx[2], bx[1], o2v)):
                nc.vector.tensor_scalar(lo[:], srcv, -1.0, 0.0,
                                        op0=OP.mult, op1=OP.max)
                nc.vector.tensor_scalar_max(hi[:], srcv, 0.0)
                nc.vector.tensor_tensor(tmp[:], lo[:], hi[:], op=OP.add)
                nc.vector.tensor_scalar(mid[:], tmp[:], -1.0, 1.0,
                                        op0=OP.mult, op1=OP.add)
                nc.vector.tensor_scalar_max(mid[:], mid[:], 0.0)
            for i in range(3):
                nc.vector.tensor_tensor(ay[i][:], ay[i][:], mask[:], op=OP.mult)
            g = [[wts.tile([128, RB, 9], F32, name="g%d%d" % (a, b),
                           tag="g%d%d" % (a, b), bufs=2)
                  for b in range(3)] for a in range(3)]
            for a in range(3):
                for b in range(3):
                    nc.vector.tensor_tensor(g[a][b][:], ay[a][:], bx[b][:],
                                            op=OP.mult)
            return g

        g_next = emit_offsets(0)
        for it in range(11, NR1 // 2):
            conv1_iter(it, pool=ps_f, ptag="pf")
        for blk in range(NBLK):
            r0b = blk * RB
            g = g_next
            if blk + 1 < NBLK:
                g_next = emit_offsets(blk + 1)

            masterT = ps_m.tile([128, RB, 128], F32, tag="masterT")
            masterD = None
            masterP = None
            # open all 4 banks with full-width zero matmuls (sets has_written
            # for the whole region; all later accumulations use start=False)
            for q in range(RB // 4):
                nc.tensor.matmul(
                    masterT[:, q * 4:(q + 1) * 4, :].rearrange(
                        "p a b -> p (a b)"),
                    c_zero[:], xsb0t[:, 0:512], start=True, stop=False)

            def acc_mm(j, lhs_ap, stop=False):
                return nc.tensor.matmul(masterT[:, j, :], lhs_ap, c_id[:],
                                        start=False, stop=stop)

            PDEPTH = int(os.environ.get('K2_PDEPTH', '2'))
            pend = []   # queue of (k, dx, ysl) awaiting term processing
            for ui in range(len(units) + PDEPTH):
                if ui < len(units):
                    k, dx = units[ui]
                    ky, kx = k // 3, k % 3
                    ysl = yp.tile([128, NYR, 128], BF16, tag="ysl")
                    t0 = 0
                    for nch, ch_rows in enumerate(YCH):
                        pt = ps_y.tile([128, ch_rows, 128], F32, tag="ypsum")
                        for tt in range(ch_rows):
                            t = t0 + tt
                            j1 = r0b + t + ky + 1
                            lhs = out1[:, j1, 1 + kx + dx:1 + kx + dx + W]
                            nc.tensor.matmul(pt[:, tt, :], lhs, c_wk[:, k, :],
                                             start=True, stop=True)
                        dst = ysl[:, t0:t0 + ch_rows, :]
                        ev = EVAC[evac_i % len(EVAC)]
                        evac_i += 1
                        if ev == 'A':
                            nc.scalar.copy(dst, pt[:])
                        else:
                            nc.vector.tensor_copy(dst, pt[:])
                        t0 += ch_rows
                    pend.append((k, dx, ysl))

                if ui >= PDEPTH:
                    k, dx, ysl = pend.pop(0)
                    ky, kx = k // 3, k % 3
                    b = dx + 1
                    for dy in (-1, 0, 1):
                        a = dy + 1
                        ti = k * 9 + (dx + 1) * 3 + (dy + 1)
                        me, ae = MULT[ti], ACC[ti]
                        gd = g[a][b]
                        srcv = ysl[:, dy + 1:dy + 1 + RB, :]

                        is_last = (ti == last_e_ti and not has_aD
                                   and not has_aP)

                        def mult_row(dst2, jj, j):
                            if me == 'A':
                                nc.scalar.mul(dst2[:, jj, :],
                                              ysl[:, j + dy + 1, :],
                                              gd[:, j, k:k + 1])
                            else:
                                nc.vector.tensor_scalar(
                                    dst2[:, jj, :], ysl[:, j + dy + 1, :],
                                    gd[:, j, k:k + 1], None, op0=OP.mult)

                        if ae == 'E' and me != 'P':
                            # fine-grained: 4-row tmps, per-row PE accum
                            for q in range(RB // 4):
                                t4 = tmpp.tile([128, 4, 128], BF16, tag="t4")
                                for jj in range(4):
                                    mult_row(t4, jj, q * 4 + jj)
                                for jj in range(4):
                                    acc_mm(q * 4 + jj, t4[:, jj, :],
                                           stop=(is_last and jj == 3))
                            continue

                        # block-granular path
                        if ae == 'D':
                            if masterD is None:
                                masterD = fin.tile([128, RB, 128], BF16,
                                                   tag="masterD", bufs=1)
                                dst, direct = masterD, True
                            else:
                                dst = tmpb.tile([128, RB, 128], BF16, tag="tB")
                                direct = False
                        elif ae == 'P':
                            if masterP is None:
                                masterP = fin.tile([128, RB, 128], BF16,
                                                   tag="masterP", bufs=1)
                                dst, direct = masterP, True
                            else:
                                dst = tmpb.tile([128, RB, 128], BF16, tag="tB")
                                direct = False
                        else:
                            dst = tmpb.tile([128, RB, 128], BF16, tag="tB")
                            direct = False

                        if me == 'P':
                            gb = gd[:, 0:RB, k:k + 1] \
                                .broadcast_to([128, RB, 128])
                            nc.gpsimd.tensor_tensor(dst[:], srcv, gb, op=OP.mult)
                        else:
                            for j in range(RB):
                                mult_row(dst, j, j)

                        if ae == 'E':
                            for j in range(RB):
                                acc_mm(j, dst[:, j, :],
                                       stop=(is_last and j % 4 == 3))
                        elif ae == 'D' and not direct:
                            nc.vector.tensor_tensor(masterD[:], masterD[:],
                                                    dst[:], op=OP.add)
                        elif ae == 'P' and not direct:
                            nc.gpsimd.tensor_tensor(masterP[:], masterP[:],
                                                    dst[:], op=OP.add)

            # fold pixel-major masters into channel-major PSUM master
            if masterD is not None:
                for j in range(RB):
                    acc_mm(j, masterD[:, j, :],
                           stop=(masterP is None and j % 4 == 3))
            if masterP is not None:
                for j in range(RB):
                    acc_mm(j, masterP[:, j, :], stop=(j % 4 == 3))

            # ---- tail: bn2+relu, conv3, bn3, downsample, final relu (ch-major)
            for q in range(RB // 4):
                out2 = fin.tile([128, 4, 128], BF16, tag="out2")
                nc.scalar.activation(out2[:], masterT[:, q * 4:(q + 1) * 4, :],
                                     AF.Relu, bias=c_bdc2[:, :], scale=1.0)
                pt3 = ps_f.tile([128, 512], F32, tag="pf")
                nc.tensor.matmul(pt3[:], c_w3[:],
                                 out2[:].rearrange("p a b -> p (a b)"),
                                 start=True, stop=True)
                a1 = work.tile([128, 512], F32, tag="a1")
                nc.scalar.activation(a1[:], pt3[:], AF.Relu, bias=c_t3a[:, :],
                                     scale=1.0)
                ptd = ps_f.tile([128, 512], F32, tag="pf")
                px0 = (r0b + 3 + q * 4) * W
                nc.tensor.matmul(ptd[:], c_wds[:, 0, :], xsb0t[:, px0:px0 + 512],
                                 start=True, stop=False)
                nc.tensor.matmul(ptd[:], c_wds[:, 1, :], xsb1t[:, px0:px0 + 512],
                                 start=False, stop=True)
                s1 = work.tile([128, 512], F32, tag="s1")
                nc.vector.scalar_tensor_tensor(s1[:], a1[:], c_s3b[:, :], ptd[:],
                                               op0=OP.mult, op1=OP.add)
                res = fin.tile([128, 512], F32, tag="res")
                nc.scalar.activation(res[:], s1[:], AF.Relu, bias=c_tfin[:, :],
                                     scale=1.0)
                nc.sync.dma_start(
                    out_d[:, (r0b + q * 4) * W:(r0b + q * 4 + 4) * W], res[:])
    return out_d


def _fold(inp):
    f32 = np.float32
    w1full = (inp['w1'] * inp['s1a'][:, None]).astype(f32)
    w1f = np.ascontiguousarray(np.stack(
        [w1full[:, h * 128:(h + 1) * 128].T for h in range(2)], axis=1)).astype(BF)
    woffT = np.ascontiguousarray(np.stack(
        [inp['w_off'][:, :, k // 3, k % 3].T for k in range(KK)], axis=1)).astype(BF)
    s2 = inp['s2']
    wk = np.ascontiguousarray(np.stack(
        [(inp['w_dc'][:, :, k // 3, k % 3] * s2[:, None]).T for k in range(KK)],
        axis=1)).astype(BF)
    bdc2 = (s2 * inp['b_dc'] + inp['t2']).astype(f32)
    w3f = np.ascontiguousarray((inp['w3'] * inp['s3a'][:, None]).T).astype(BF)
    b_dsf = (inp['sd'] * inp['b_ds'] + inp['td']).astype(f32)
    wdsfull = (inp['w_ds'] * inp['sd'][:, None]).astype(f32)
    wdsf = np.ascontiguousarray(np.stack(
        [wdsfull[:, h * 128:(h + 1) * 128].T for h in range(2)], axis=1)).astype(BF)
    col = lambda v: np.ascontiguousarray(np.asarray(v, f32).reshape(-1, 1))
    return {
        'w1f': w1f, 't1a': col(inp['t1a']), 's1b': col(inp['s1b']),
        't1b': col(inp['t1b']), 'woffT': woffT, 'b_off': col(inp['b_off']),
        'wk': wk, 'iden': np.eye(128, dtype=BF),
        'bdc2c': col(bdc2),
        'w3f': w3f, 't3a': col(inp['t3a']), 's3b': col(inp['s3b']),
        'tfin': col(inp['t3b'] + b_dsf), 'wdsf': wdsf,
    }


def _x_slab(x_b, r0):
    sl = np.zeros((256, NR1, W), np.float32)
    lo, hi = r0 - MARG, r0 + ROWS_OUT + MARG
    slo, shi = max(lo, 0), min(hi, H)
    sl[:, slo - lo:shi - lo, :] = x_b[:, slo:shi, :]
    return np.ascontiguousarray(sl.reshape(2, 128, NR1 * W)).astype(BF)


_CACHE = {}


def kernel(**inputs):
    inp = {k: np.asarray(v) for k, v in inputs.items()}
    shared = _fold(inp)
    in_maps = []
    for core in range(N_CORES):
        b, half = core // 2, core % 2
        m = dict(shared)
        m['xs'] = _x_slab(inp['x'][b], half * ROWS_OUT)
        in_maps.append(m)
    if 'nc' not in _CACHE:
        nc = bacc.Bacc()
        _build(nc)
        nc.compile()
        _CACHE['nc'] = nc
    nc = _CACHE['nc']
    res = run_bass_kernel_spmd(nc, in_maps, core_ids=list(range(N_CORES)))
    out = np.zeros((B, PL, H, W), np.float32)
    for core in range(N_CORES):
        b, half = core // 2, core % 2
        r0 = half * ROWS_OUT
        out[b, :, r0:r0 + ROWS_OUT, :] = np.asarray(
            res.results[core]['out'], np.float32).reshape(128, ROWS_OUT, W)
    return out


if __name__ == "__main__":
    pass


# revision 7
# speedup vs baseline: 1.0158x; 1.0158x over previous
"""Trainium2 Bass kernel for nn_DeformBottleneck (DCNv2 bottleneck block), v2.

Same sharding as v1: 8 shards = (batch b, row-half) on 8 cores; each core
computes output rows [r0, r0+64) of one image.

v2 restructure vs v1:
  - Y-field matmuls write fp32 PSUM in small chunks; evacuation split
    across DVE and ACT with a deep chunk ring.
  - The 81 hat-stencil weight terms per row are TS-mults at 4x on DVE
    (per-row [x,ch] tiles, per-partition scalar), with overflow terms on
    ACT (activation scale) and POOL (broadcast tensor_tensor blocks).
  - Accumulation goes to the Tensor engine: weighted tmp tiles transpose-
    accumulate (matmul vs identity) into a channel-major fp32 PSUM master
    [ch, 16, W] per block; leftover terms accumulate into pixel-major
    bf16 masters on DVE/POOL, folded into the PSUM master at block end.
  - Channel-major master kills the conv3 transposes: bn2+relu / conv3 /
    bn3 / downsample / final relu all run channel-major.
"""

import os
import sys
from contextlib import ExitStack

import numpy as np

sys.path.insert(0, "/opt/trn_rl_repo")

import ml_dtypes

import concourse.bass as bass
from concourse import bacc
import concourse.mybir as mybir
import concourse.tile as tile
from concourse.bass_utils import run_bass_kernel_spmd

BF = ml_dtypes.bfloat16
F32 = mybir.dt.float32
BF16 = mybir.dt.bfloat16
I32 = mybir.dt.int32
AF = mybir.ActivationFunctionType
OP = mybir.AluOpType

B, CIN, H, W = 4, 256, 128, 128
PL, KK = 128, 9
PW = 132          # padded out1 slab width
ROWS_OUT = 64     # output rows per core
MARG = 3
NR1 = ROWS_OUT + 2 * MARG
RB = 16           # rows per block
NBLK = ROWS_OUT // RB
NYR = RB + 2
N_CORES = 8

# chunking of the 18-row Y field into PSUM pieces
YCH = (4, 4, 4, 4, 2)


def _sched(weights, n):
    """Interleaved largest-remainder schedule: n picks from weighted set."""
    tot = float(sum(weights.values()))
    acc = {e: 0.0 for e in weights}
    out = []
    for _ in range(n):
        for e in acc:
            acc[e] += weights[e] / tot
        pick = max(acc, key=lambda e: (acc[e], e))
        acc[pick] -= 1.0
        out.append(pick)
    return out


def _parse_w(s, default):
    # "D52A10P19" -> {'D':52,'A':10,'P':19}
    if not s:
        return default
    out = {}
    key = None
    num = ''
    for ch in s:
        if ch.isalpha():
            if key is not None:
                out[key] = int(num)
            key = ch
            num = ''
        else:
            num += ch
    out[key] = int(num)
    return out


def _build(nc):
    MULT_W = _parse_w(os.environ.get('K2_MULT', ''), {'D': 55, 'A': 8, 'P': 18})
    ACC_W = _parse_w(os.environ.get('K2_ACC', ''), {'E': 66, 'D': 8, 'P': 7})
    # per-term engine assignment, term order: (k-major, dx, dy)
    MULT = _sched(MULT_W, 81)
    ACC = _sched(ACC_W, 81)
    if os.environ.get('K2_ACCSORT'):
        ACC = sorted(ACC, key=lambda e: (e == 'E'))
    EVAC = _sched(_parse_w(os.environ.get('K2_EVAC', ''), {'D': 1, 'A': 4}), 60)

    def di(name, shape, dt=F32):
        return nc.dram_tensor(name, shape, dt, kind="ExternalInput")

    xs = di("xs", [2, 128, NR1 * W], BF16)
    w1f = di("w1f", [128, 2, 128], BF16)
    t1a = di("t1a", [128, 1])
    s1b = di("s1b", [128, 1])
    t1b = di("t1b", [128, 1])
    woffT = di("woffT", [128, KK, 27], BF16)
    b_off = di("b_off", [27, 1])
    wk = di("wk", [128, KK, 128], BF16)
    iden = di("iden", [128, 128], BF16)
    bdc2c = di("bdc2c", [128, 1])
    w3f = di("w3f", [128, 128], BF16)
    t3a = di("t3a", [128, 1])
    s3b = di("s3b", [128, 1])
    tfin = di("tfin", [128, 1])
    wdsf = di("wdsf", [128, 2, 128], BF16)
    out_d = nc.dram_tensor("out", [128, ROWS_OUT * W], F32, kind="ExternalOutput")

    with tile.TileContext(nc) as tc, ExitStack() as ctx:
        P = lambda name, bufs=1, **kw: ctx.enter_context(
            tc.tile_pool(name=name, bufs=bufs, **kw))
        consts = P("consts")
        big = P("big")
        wts = P("wts")
        work = P("work", bufs=2)

        c_w1 = consts.tile([128, 2, 128], BF16); nc.sync.dma_start(c_w1[:], w1f[:])
        c_t1a = consts.tile([128, 1], F32); nc.sync.dma_start(c_t1a[:], t1a[:])
        c_s1b = consts.tile([128, 1], F32); nc.sync.dma_start(c_s1b[:], s1b[:])
        c_t1b = consts.tile([128, 1], F32); nc.sync.dma_start(c_t1b[:], t1b[:])
        c_woff = consts.tile([128, KK, 27], BF16); nc.sync.dma_start(c_woff[:], woffT[:])
        c_boff = consts.tile([27, 1], F32); nc.sync.dma_start(c_boff[:], b_off[:])
        c_wk = consts.tile([128, KK, 128], BF16); nc.sync.dma_start(c_wk[:], wk[:])
        c_id = consts.tile([128, 128], BF16); nc.sync.dma_start(c_id[:], iden[:])
        c_zero = consts.tile([128, 128], BF16); nc.vector.memset(c_zero[:], 0.0)
        c_bdc2 = consts.tile([128, 1], F32); nc.sync.dma_start(c_bdc2[:], bdc2c[:])
        c_w3 = consts.tile([128, 128], BF16); nc.sync.dma_start(c_w3[:], w3f[:])
        c_t3a = consts.tile([128, 1], F32); nc.sync.dma_start(c_t3a[:], t3a[:])
        c_s3b = consts.tile([128, 1], F32); nc.sync.dma_start(c_s3b[:], s3b[:])
        c_tfin = consts.tile([128, 1], F32); nc.sync.dma_start(c_tfin[:], tfin[:])
        c_wds = consts.tile([128, 2, 128], BF16); nc.sync.dma_start(c_wds[:], wdsf[:])

        xsb0t = big.tile([128, NR1 * W], BF16)
        xsb1t = big.tile([128, NR1 * W], BF16)
        nc.sync.dma_start(xsb0t[:], xs[0])
        nc.sync.dma_start(xsb1t[:], xs[1])

        out1 = big.tile([128, NR1, PW], BF16)
        nc.gpsimd.memset(out1[:], 0.0)

        pre_ctx = tc.tile_pool(name="ps_pre", bufs=2, space="PSUM")
        ps_a = pre_ctx.__enter__()

        # ---- conv1 (1x1 256->128) + BN + relu, twice-relu'd -> out1 slab
        def conv1_iter(it, pool=None, ptag="c1"):
            px0 = it * 2 * W
            pt = (pool or ps_a).tile([128, 2, 128], F32, tag=ptag)
            nc.tensor.matmul(pt[:], c_w1[:, 0, :], xsb0t[:, px0:px0 + 256],
                             start=True, stop=False)
            nc.tensor.matmul(pt[:], c_w1[:, 1, :], xsb1t[:, px0:px0 + 256],
                             start=False, stop=True)
            t = work.tile([128, 2, 128], F32, tag="c1s")
            nc.scalar.activation(t[:], pt[:], AF.Relu, bias=c_t1a[:, :], scale=1.0)
            nc.vector.tensor_scalar(t[:], t[:], c_s1b[:, :], c_t1b[:, :],
                                    op0=OP.mult, op1=OP.add)
            nc.vector.tensor_scalar_max(out1[:, it * 2:it * 2 + 2, 2:130], t[:], 0.0)

        for it in range(11):
            conv1_iter(it)

        pre_ctx.__exit__(None, None, None)
        offp = ctx.enter_context(tc.tile_pool(name="offp", bufs=2))

        ps_y = ctx.enter_context(tc.tile_pool(name="ps_y", bufs=int(os.environ.get("K2_PSYB","3")), space="PSUM"))
        ps_m = ctx.enter_context(tc.tile_pool(name="ps_m", bufs=1, space="PSUM"))
        ps_f = ctx.enter_context(tc.tile_pool(name="ps_f", bufs=1, space="PSUM"))
        yp = ctx.enter_context(tc.tile_pool(name="yp", bufs=int(os.environ.get("K2_YB","5"))))
        tmpp = ctx.enter_context(tc.tile_pool(name="tmpp", bufs=int(os.environ.get("K2_TB","24"))))
        tmpb = ctx.enter_context(tc.tile_pool(name="tmpb", bufs=int(os.environ.get("K2_TBB","4"))))
        fin = ctx.enter_context(tc.tile_pool(name="fin", bufs=2))

        # which accumulate-engines appear (for stop-flag placement)
        has_aD = 'D' in ACC
        has_aP = 'P' in ACC
        last_e_ti = max((ti for ti in range(81) if ACC[ti] == 'E'), default=-1)

        # ---- main loop: per block, stream (k,dx) Y-fields, weight + accum
        units = [(k, dx) for k in range(9) for dx in (-1, 0, 1)]
        evac_i = 0
        def emit_offsets(blk):
            # offsets for block blk: conv, transpose, hat weights g[a][b]
            r0b = blk * RB
            off = offp.tile([27, RB * W], BF16, tag="off")
            for it in range(RB // 4):
                r0 = r0b + it * 4
                pt = ps_f.tile([27, 512], F32, tag="pf")
                for k in range(KK):
                    ky, kx = k // 3, k % 3
                    src = out1[:, r0 + ky + 2:r0 + ky + 6, 1 + kx:1 + kx + W]
                    nc.tensor.matmul(pt[:], c_woff[:, k, :], src,
                                     start=(k == 0), stop=(k == KK - 1))
                nc.scalar.activation(off[:, it * 4 * W:(it + 1) * 4 * W], pt[:],
                                     AF.Identity, bias=c_boff[:, :], scale=1.0)
            offT = offp.tile([128, RB, 28], F32, tag="offT")
            pt = ps_f.tile([128, RB, 28], BF16, tag="pf")
            for j in range(RB):
                nc.tensor.transpose(pt[:, j, 0:27], off[:, j * W:(j + 1) * W],
                                    c_id[0:27, 0:27])
            nc.vector.tensor_copy(offT[:, :, 0:27], pt[:, :, 0:27])

            o1v, o2v, o3v = (offT[:, :, 0:9], offT[:, :, 9:18],
                             offT[:, :, 18:27])
            mask = offp.tile([128, RB, 9], BF16, tag="mask")
            nc.scalar.activation(mask[:], o3v, AF.Sigmoid)
            ay = [offp.tile([128, RB, 9], BF16, name="ayt%d" % i,
                            tag="ayt" + str(i)) for i in range(3)]
            bx = [offp.tile([128, RB, 9], BF16, name="bxt%d" % i,
                            tag="bxt" + str(i)) for i in range(3)]
            tmp = offp.tile([128, RB, 9], BF16, tag="tmp9")
            for (lo, hi, mid, srcv) in ((ay[0], ay[2], ay[1], o1v),
                                        (bx[0], bx[2], bx[1], o2v)):
                nc.vector.tensor_scalar(lo[:], srcv, -1.0, 0.0,
                                        op0=OP.mult, op1=OP.max)
                nc.vector.tensor_scalar_max(hi[:], srcv, 0.0)
                nc.vector.tensor_tensor(tmp[:], lo[:], hi[:], op=OP.add)
                nc.vector.tensor_scalar(mid[:], tmp[:], -1.0, 1.0,
                                        op0=OP.mult, op1=OP.add)
                nc.vector.tensor_scalar_max(mid[:], mid[:], 0.0)
            for i in range(3):
                nc.vector.tensor_tensor(ay[i][:], ay[i][:], mask[:], op=OP.mult)
            g = [[wts.tile([128, RB, 9], F32, name="g%d%d" % (a, b),
                           tag="g%d%d" % (a, b), bufs=2)
                  for b in range(3)] for a in range(3)]
            for a in range(3):
                for b in range(3):
                    nc.vector.tensor_tensor(g[a][b][:], ay[a][:], bx[b][:],
                                            op=OP.mult)
            return g

        g_next = emit_offsets(0)
        for it in range(11, NR1 // 2):
            conv1_iter(it, pool=ps_f, ptag="pf")
        for blk in range(NBLK):
            r0b = blk * RB
            g = g_next
            if blk + 1 < NBLK:
                g_next = emit_offsets(blk + 1)

            masterT = ps_m.tile([128, RB, 128], F32, tag="masterT")
            masterD = None
            masterP = None
            # open all 4 banks with full-width zero matmuls (sets has_written
            # for the whole region; all later accumulations use start=False)
            for q in range(RB // 4):
                nc.tensor.matmul(
                    masterT[:, q * 4:(q + 1) * 4, :].rearrange(
                        "p a b -> p (a b)"),
                    c_zero[:], xsb0t[:, 0:512], start=True, stop=False)

            def acc_mm(j, lhs_ap, stop=False):
                return nc.tensor.matmul(masterT[:, j, :], lhs_ap, c_id[:],
                                        start=False, stop=stop)

            PDEPTH = int(os.environ.get('K2_PDEPTH', '2'))
            pend = []   # queue of (k, dx, ysl) awaiting term processing
            for ui in range(len(units) + PDEPTH):
                if ui < len(units):
                    k, dx = units[ui]
                    ky, kx = k // 3, k % 3
                    ysl = yp.tile([128, NYR, 128], BF16, tag="ysl")
                    t0 = 0
                    for nch, ch_rows in enumerate(YCH):
                        pt = ps_y.tile([128, ch_rows, 128], F32, tag="ypsum")
                        for tt in range(ch_rows):
                            t = t0 + tt
                            j1 = r0b + t + ky + 1
                            lhs = out1[:, j1, 1 + kx + dx:1 + kx + dx + W]
                            nc.tensor.matmul(pt[:, tt, :], lhs, c_wk[:, k, :],
                                             start=True, stop=True)
                        dst = ysl[:, t0:t0 + ch_rows, :]
                        ev = EVAC[evac_i % len(EVAC)]
                        evac_i += 1
                        if ev == 'A':
                            nc.scalar.copy(dst, pt[:])
                        else:
                            nc.vector.tensor_copy(dst, pt[:])
                        t0 += ch_rows
                    pend.append((k, dx, ysl))

                if ui >= PDEPTH:
                    k, dx, ysl = pend.pop(0)
                    ky, kx = k // 3, k % 3
                    b = dx + 1
                    for dy in (-1, 0, 1):
                        a = dy + 1
                        ti = k * 9 + (dx + 1) * 3 + (dy + 1)
                        me, ae = MULT[ti], ACC[ti]
                        gd = g[a][b]
                        srcv = ysl[:, dy + 1:dy + 1 + RB, :]

                        is_last = (ti == last_e_ti and not has_aD
                                   and not has_aP)

                        def mult_row(dst2, jj, j):
                            if me == 'A':
                                nc.scalar.mul(dst2[:, jj, :],
                                              ysl[:, j + dy + 1, :],
                                              gd[:, j, k:k + 1])
                            else:
                                nc.vector.tensor_scalar(
                                    dst2[:, jj, :], ysl[:, j + dy + 1, :],
                                    gd[:, j, k:k + 1], None, op0=OP.mult)

                        if ae == 'E' and me != 'P':
                            # fine-grained: 4-row tmps, per-row PE accum
                            for q in range(RB // 4):
                                t4 = tmpp.tile([128, 4, 128], BF16, tag="t4")
                                for jj in range(4):
                                    mult_row(t4, jj, q * 4 + jj)
                                for jj in range(4):
                                    acc_mm(q * 4 + jj, t4[:, jj, :],
                                           stop=(is_last and jj == 3))
                            continue

                        # block-granular path
                        if ae == 'D':
                            if masterD is None:
                                masterD = fin.tile([128, RB, 128], BF16,
                                                   tag="masterD", bufs=1)
                                dst, direct = masterD, True
                            else:
                                dst = tmpb.tile([128, RB, 128], BF16, tag="tB")
                                direct = False
                        elif ae == 'P':
                            if masterP is None:
                                masterP = fin.tile([128, RB, 128], BF16,
                                                   tag="masterP", bufs=1)
                                dst, direct = masterP, True
                            else:
                                dst = tmpb.tile([128, RB, 128], BF16, tag="tB")
                                direct = False
                        else:
                            dst = tmpb.tile([128, RB, 128], BF16, tag="tB")
                            direct = False

                        if me == 'P':
                            H2 = RB // int(os.environ.get('K2_PSPLIT', '1'))
                            for h0 in range(0, RB, H2):
                                gb = gd[:, h0:h0 + H2, k:k + 1] \
                                    .broadcast_to([128, H2, 128])
                                nc.gpsimd.tensor_tensor(
                                    dst[:, h0:h0 + H2, :],
                                    ysl[:, dy + 1 + h0:dy + 1 + h0 + H2, :],
                                    gb, op=OP.mult)
                        else:
                            for j in range(RB):
                                mult_row(dst, j, j)

                        if ae == 'E':
                            for j in range(RB):
                                acc_mm(j, dst[:, j, :],
                                       stop=(is_last and j % 4 == 3))
                        elif ae == 'D' and not direct:
                            nc.vector.tensor_tensor(masterD[:], masterD[:],
                                                    dst[:], op=OP.add)
                        elif ae == 'P' and not direct:
                            H2 = RB // int(os.environ.get('K2_PSPLIT', '1'))
                            for h0 in range(0, RB, H2):
                                nc.gpsimd.tensor_tensor(
                                    masterP[:, h0:h0 + H2, :],
                                    masterP[:, h0:h0 + H2, :],
                                    dst[:, h0:h0 + H2, :], op=OP.add)

            # fold pixel-major masters into channel-major PSUM master
            if masterD is not None:
                for j in range(RB):
                    acc_mm(j, masterD[:, j, :],
                           stop=(masterP is None and j % 4 == 3))
            if masterP is not None:
                for j in range(RB):
                    acc_mm(j, masterP[:, j, :], stop=(j % 4 == 3))

            # ---- tail: bn2+relu, conv3, bn3, downsample, final relu (ch-major)
            for q in range(RB // 4):
                out2 = fin.tile([128, 4, 128], BF16, tag="out2")
                nc.scalar.activation(out2[:], masterT[:, q * 4:(q + 1) * 4, :],
                                     AF.Relu, bias=c_bdc2[:, :], scale=1.0)
                pt3 = ps_f.tile([128, 512], F32, tag="pf")
                nc.tensor.matmul(pt3[:], c_w3[:],
                                 out2[:].rearrange("p a b -> p (a b)"),
                                 start=True, stop=True)
                a1 = work.tile([128, 512], F32, tag="a1")
                nc.scalar.activation(a1[:], pt3[:], AF.Relu, bias=c_t3a[:, :],
                                     scale=1.0)
                ptd = ps_f.tile([128, 512], F32, tag="pf")
                px0 = (r0b + 3 + q * 4) * W
                nc.tensor.matmul(ptd[:], c_wds[:, 0, :], xsb0t[:, px0:px0 + 512],
                                 start=True, stop=False)
                nc.tensor.matmul(ptd[:], c_wds[:, 1, :], xsb1t[:, px0:px0 + 512],
                                 start=False, stop=True)
                s1 = work.tile([128, 512], F32, tag="s1")
                nc.vector.scalar_tensor_tensor(s1[:], a1[:], c_s3b[:, :], ptd[:],
                                               op0=OP.mult, op1=OP.add)
                res = fin.tile([128, 512], F32, tag="res")
                nc.scalar.activation(res[:], s1[:], AF.Relu, bias=c_tfin[:, :],
                                     scale=1.0)
                nc.sync.dma_start(
                    out_d[:, (r0b + q * 4) * W:(r0b + q * 4 + 4) * W], res[:])
    return out_d


def _fold(inp):
    f32 = np.float32
    w1full = (inp['w1'] * inp['s1a'][:, None]).astype(f32)
    w1f = np.ascontiguousarray(np.stack(
        [w1full[:, h * 128:(h + 1) * 128].T for h in range(2)], axis=1)).astype(BF)
    woffT = np.ascontiguousarray(np.stack(
        [inp['w_off'][:, :, k // 3, k % 3].T for k in range(KK)], axis=1)).astype(BF)
    s2 = inp['s2']
    wk = np.ascontiguousarray(np.stack(
        [(inp['w_dc'][:, :, k // 3, k % 3] * s2[:, None]).T for k in range(KK)],
        axis=1)).astype(BF)
    bdc2 = (s2 * inp['b_dc'] + inp['t2']).astype(f32)
    w3f = np.ascontiguousarray((inp['w3'] * inp['s3a'][:, None]).T).astype(BF)
    b_dsf = (inp['sd'] * inp['b_ds'] + inp['td']).astype(f32)
    wdsfull = (inp['w_ds'] * inp['sd'][:, None]).astype(f32)
    wdsf = np.ascontiguousarray(np.stack(
        [wdsfull[:, h * 128:(h + 1) * 128].T for h in range(2)], axis=1)).astype(BF)
    col = lambda v: np.ascontiguousarray(np.asarray(v, f32).reshape(-1, 1))
    return {
        'w1f': w1f, 't1a': col(inp['t1a']), 's1b': col(inp['s1b']),
        't1b': col(inp['t1b']), 'woffT': woffT, 'b_off': col(inp['b_off']),
        'wk': wk, 'iden': np.eye(128, dtype=BF),
        'bdc2c': col(bdc2),
        'w3f': w3f, 't3a': col(inp['t3a']), 's3b': col(inp['s3b']),
        'tfin': col(inp['t3b'] + b_dsf), 'wdsf': wdsf,
    }


def _x_slab(x_b, r0):
    sl = np.zeros((256, NR1, W), np.float32)
    lo, hi = r0 - MARG, r0 + ROWS_OUT + MARG
    slo, shi = max(lo, 0), min(hi, H)
    sl[:, slo - lo:shi - lo, :] = x_b[:, slo:shi, :]
    return np.ascontiguousarray(sl.reshape(2, 128, NR1 * W)).astype(BF)


_CACHE = {}


def kernel(**inputs):
    inp = {k: np.asarray(v) for k, v in inputs.items()}
    shared = _fold(inp)
    in_maps = []
    for core in range(N_CORES):
        b, half = core // 2, core % 2
        m = dict(shared)
        m['xs'] = _x_slab(inp['x'][b], half * ROWS_OUT)
        in_maps.append(m)
    if 'nc' not in _CACHE:
        nc = bacc.Bacc()
        _build(nc)
        nc.compile()
        _CACHE['nc'] = nc
    nc = _CACHE['nc']
    res = run_bass_kernel_spmd(nc, in_maps, core_ids=list(range(N_CORES)))
    out = np.zeros((B, PL, H, W), np.float32)
    for core in range(N_CORES):
        b, half = core // 2, core % 2
        r0 = half * ROWS_OUT
        out[b, :, r0:r0 + ROWS_OUT, :] = np.asarray(
            res.results[core]['out'], np.float32).reshape(128, ROWS_OUT, W)
    return out


if __name__ == "__main__":
    pass


# revision 8
# speedup vs baseline: 1.0183x; 1.0024x over previous
"""Trainium2 Bass kernel for nn_DeformBottleneck (DCNv2 bottleneck block), v2.

Same sharding as v1: 8 shards = (batch b, row-half) on 8 cores; each core
computes output rows [r0, r0+64) of one image.

v2 restructure vs v1:
  - Y-field matmuls write fp32 PSUM in small chunks; evacuation split
    across DVE and ACT with a deep chunk ring.
  - The 81 hat-stencil weight terms per row are TS-mults at 4x on DVE
    (per-row [x,ch] tiles, per-partition scalar), with overflow terms on
    ACT (activation scale) and POOL (broadcast tensor_tensor blocks).
  - Accumulation goes to the Tensor engine: weighted tmp tiles transpose-
    accumulate (matmul vs identity) into a channel-major fp32 PSUM master
    [ch, 16, W] per block; leftover terms accumulate into pixel-major
    bf16 masters on DVE/POOL, folded into the PSUM master at block end.
  - Channel-major master kills the conv3 transposes: bn2+relu / conv3 /
    bn3 / downsample / final relu all run channel-major.
"""

import os
import sys
from contextlib import ExitStack

import numpy as np

sys.path.insert(0, "/opt/trn_rl_repo")

import ml_dtypes

import concourse.bass as bass
from concourse import bacc
import concourse.mybir as mybir
import concourse.tile as tile
from concourse.bass_utils import run_bass_kernel_spmd

BF = ml_dtypes.bfloat16
F32 = mybir.dt.float32
BF16 = mybir.dt.bfloat16
I32 = mybir.dt.int32
AF = mybir.ActivationFunctionType
OP = mybir.AluOpType

B, CIN, H, W = 4, 256, 128, 128
PL, KK = 128, 9
PW = 132          # padded out1 slab width
ROWS_OUT = 64     # output rows per core
MARG = 3
NR1 = ROWS_OUT + 2 * MARG
RB = 16           # rows per block
NBLK = ROWS_OUT // RB
NYR = RB + 2
N_CORES = 8

# chunking of the 18-row Y field into PSUM pieces
YCH = (4, 4, 4, 4, 2)


def _sched(weights, n):
    """Interleaved largest-remainder schedule: n picks from weighted set."""
    tot = float(sum(weights.values()))
    acc = {e: 0.0 for e in weights}
    out = []
    for _ in range(n):
        for e in acc:
            acc[e] += weights[e] / tot
        pick = max(acc, key=lambda e: (acc[e], e))
        acc[pick] -= 1.0
        out.append(pick)
    return out


def _parse_w(s, default):
    # "D52A10P19" -> {'D':52,'A':10,'P':19}
    if not s:
        return default
    out = {}
    key = None
    num = ''
    for ch in s:
        if ch.isalpha():
            if key is not None:
                out[key] = int(num)
            key = ch
            num = ''
        else:
            num += ch
    out[key] = int(num)
    return out


def _build(nc):
    MULT_W = _parse_w(os.environ.get('K2_MULT', ''), {'D': 55, 'A': 8, 'P': 18})
    ACC_W = _parse_w(os.environ.get('K2_ACC', ''), {'E': 66, 'D': 8, 'P': 7})
    # per-term engine assignment, term order: (k-major, dx, dy)
    MULT = _sched(MULT_W, 81)
    ACC = _sched(ACC_W, 81)
    if os.environ.get('K2_ACCSORT'):
        ACC = sorted(ACC, key=lambda e: (e == 'E'))
    EVAC = _sched(_parse_w(os.environ.get('K2_EVAC', ''), {'D': 1, 'A': 4}), 60)

    def di(name, shape, dt=F32):
        return nc.dram_tensor(name, shape, dt, kind="ExternalInput")

    xs = di("xs", [2, 128, NR1 * W], BF16)
    w1f = di("w1f", [128, 2, 128], BF16)
    t1a = di("t1a", [128, 1])
    s1b = di("s1b", [128, 1])
    t1b = di("t1b", [128, 1])
    woffT = di("woffT", [128, KK, 27], BF16)
    b_off = di("b_off", [27, 1])
    wk = di("wk", [128, KK, 128], BF16)
    iden = di("iden", [128, 128], BF16)
    bdc2c = di("bdc2c", [128, 1])
    w3f = di("w3f", [128, 128], BF16)
    t3a = di("t3a", [128, 1])
    s3b = di("s3b", [128, 1])
    tfin = di("tfin", [128, 1])
    wdsf = di("wdsf", [128, 2, 128], BF16)
    out_d = nc.dram_tensor("out", [128, ROWS_OUT * W], F32, kind="ExternalOutput")

    with tile.TileContext(nc) as tc, ExitStack() as ctx:
        P = lambda name, bufs=1, **kw: ctx.enter_context(
            tc.tile_pool(name=name, bufs=bufs, **kw))
        consts = P("consts")
        big = P("big")
        wts = P("wts")
        work = P("work", bufs=2)

        c_w1 = consts.tile([128, 2, 128], BF16); nc.sync.dma_start(c_w1[:], w1f[:])
        c_t1a = consts.tile([128, 1], F32); nc.sync.dma_start(c_t1a[:], t1a[:])
        c_s1b = consts.tile([128, 1], F32); nc.sync.dma_start(c_s1b[:], s1b[:])
        c_t1b = consts.tile([128, 1], F32); nc.sync.dma_start(c_t1b[:], t1b[:])
        c_woff = consts.tile([128, KK, 27], BF16); nc.sync.dma_start(c_woff[:], woffT[:])
        c_boff = consts.tile([27, 1], F32); nc.sync.dma_start(c_boff[:], b_off[:])
        c_wk = consts.tile([128, KK, 128], BF16); nc.sync.dma_start(c_wk[:], wk[:])
        c_id = consts.tile([128, 128], BF16); nc.sync.dma_start(c_id[:], iden[:])
        c_zero = consts.tile([128, 128], BF16); nc.vector.memset(c_zero[:], 0.0)
        c_bdc2 = consts.tile([128, 1], F32); nc.sync.dma_start(c_bdc2[:], bdc2c[:])
        c_w3 = consts.tile([128, 128], BF16); nc.sync.dma_start(c_w3[:], w3f[:])
        c_t3a = consts.tile([128, 1], F32); nc.sync.dma_start(c_t3a[:], t3a[:])
        c_s3b = consts.tile([128, 1], F32); nc.sync.dma_start(c_s3b[:], s3b[:])
        c_tfin = consts.tile([128, 1], F32); nc.sync.dma_start(c_tfin[:], tfin[:])
        c_wds = consts.tile([128, 2, 128], BF16); nc.sync.dma_start(c_wds[:], wdsf[:])

        xsb0t = big.tile([128, NR1 * W], BF16)
        xsb1t = big.tile([128, NR1 * W], BF16)
        NQ = NR1 * W // 4
        for q in range(4):
            nc.sync.dma_start(xsb0t[:, q * NQ:(q + 1) * NQ],
                              xs[0, :, q * NQ:(q + 1) * NQ])
            nc.sync.dma_start(xsb1t[:, q * NQ:(q + 1) * NQ],
                              xs[1, :, q * NQ:(q + 1) * NQ])

        out1 = big.tile([128, NR1, PW], BF16)
        nc.gpsimd.memset(out1[:, :, 0:2], 0.0)
        nc.gpsimd.memset(out1[:, :, 130:132], 0.0)

        pre_ctx = tc.tile_pool(name="ps_pre", bufs=2, space="PSUM")
        ps_a = pre_ctx.__enter__()

        # ---- conv1 (1x1 256->128) + BN + relu, twice-relu'd -> out1 slab
        def conv1_iter(it, pool=None, ptag="c1"):
            px0 = it * 2 * W
            pt = (pool or ps_a).tile([128, 2, 128], F32, tag=ptag)
            nc.tensor.matmul(pt[:], c_w1[:, 0, :], xsb0t[:, px0:px0 + 256],
                             start=True, stop=False)
            nc.tensor.matmul(pt[:], c_w1[:, 1, :], xsb1t[:, px0:px0 + 256],
                             start=False, stop=True)
            t = work.tile([128, 2, 128], F32, tag="c1s")
            nc.scalar.activation(t[:], pt[:], AF.Relu, bias=c_t1a[:, :], scale=1.0)
            nc.vector.tensor_scalar(t[:], t[:], c_s1b[:, :], c_t1b[:, :],
                                    op0=OP.mult, op1=OP.add)
            nc.vector.tensor_scalar_max(out1[:, it * 2:it * 2 + 2, 2:130], t[:], 0.0)

        for it in range(11):
            conv1_iter(it)

        pre_ctx.__exit__(None, None, None)
        offp = ctx.enter_context(tc.tile_pool(name="offp", bufs=2))

        ps_y = ctx.enter_context(tc.tile_pool(name="ps_y", bufs=int(os.environ.get("K2_PSYB","3")), space="PSUM"))
        ps_m = ctx.enter_context(tc.tile_pool(name="ps_m", bufs=1, space="PSUM"))
        ps_f = ctx.enter_context(tc.tile_pool(name="ps_f", bufs=1, space="PSUM"))
        yp = ctx.enter_context(tc.tile_pool(name="yp", bufs=int(os.environ.get("K2_YB","5"))))
        tmpp = ctx.enter_context(tc.tile_pool(name="tmpp", bufs=int(os.environ.get("K2_TB","24"))))
        tmpb = ctx.enter_context(tc.tile_pool(name="tmpb", bufs=int(os.environ.get("K2_TBB","4"))))
        fin = ctx.enter_context(tc.tile_pool(name="fin", bufs=2))

        # which accumulate-engines appear (for stop-flag placement)
        has_aD = 'D' in ACC
        has_aP = 'P' in ACC
        last_e_ti = max((ti for ti in range(81) if ACC[ti] == 'E'), default=-1)

        # ---- main loop: per block, stream (k,dx) Y-fields, weight + accum
        units = [(k, dx) for k in range(9) for dx in (-1, 0, 1)]
        evac_i = 0
        def emit_offsets(blk):
            # offsets for block blk: conv, transpose, hat weights g[a][b]
            r0b = blk * RB
            off = offp.tile([27, RB * W], BF16, tag="off")
            for it in range(RB // 4):
                r0 = r0b + it * 4
                pt = ps_f.tile([27, 512], F32, tag="pf")
                for k in range(KK):
                    ky, kx = k // 3, k % 3
                    src = out1[:, r0 + ky + 2:r0 + ky + 6, 1 + kx:1 + kx + W]
                    nc.tensor.matmul(pt[:], c_woff[:, k, :], src,
                                     start=(k == 0), stop=(k == KK - 1))
                nc.scalar.activation(off[:, it * 4 * W:(it + 1) * 4 * W], pt[:],
                                     AF.Identity, bias=c_boff[:, :], scale=1.0)
            offT = offp.tile([128, RB, 28], F32, tag="offT")
            pt = ps_f.tile([128, RB, 28], BF16, tag="pf")
            for j in range(RB):
                nc.tensor.transpose(pt[:, j, 0:27], off[:, j * W:(j + 1) * W],
                                    c_id[0:27, 0:27])
            nc.vector.tensor_copy(offT[:, :, 0:27], pt[:, :, 0:27])

            o1v, o2v, o3v = (offT[:, :, 0:9], offT[:, :, 9:18],
                             offT[:, :, 18:27])
            mask = offp.tile([128, RB, 9], BF16, tag="mask")
            nc.scalar.activation(mask[:], o3v, AF.Sigmoid)
            ay = [offp.tile([128, RB, 9], BF16, name="ayt%d" % i,
                            tag="ayt" + str(i)) for i in range(3)]
            bx = [offp.tile([128, RB, 9], BF16, name="bxt%d" % i,
                            tag="bxt" + str(i)) for i in range(3)]
            tmp = offp.tile([128, RB, 9], BF16, tag="tmp9")
            for (lo, hi, mid, srcv) in ((ay[0], ay[2], ay[1], o1v),
                                        (bx[0], bx[2], bx[1], o2v)):
                nc.vector.tensor_scalar(lo[:], srcv, -1.0, 0.0,
                                        op0=OP.mult, op1=OP.max)
                nc.vector.tensor_scalar_max(hi[:], srcv, 0.0)
                nc.vector.tensor_tensor(tmp[:], lo[:], hi[:], op=OP.add)
                nc.vector.tensor_scalar(mid[:], tmp[:], -1.0, 1.0,
                                        op0=OP.mult, op1=OP.add)
                nc.vector.tensor_scalar_max(mid[:], mid[:], 0.0)
            for i in range(3):
                nc.vector.tensor_tensor(ay[i][:], ay[i][:], mask[:], op=OP.mult)
            g = [[wts.tile([128, RB, 9], F32, name="g%d%d" % (a, b),
                           tag="g%d%d" % (a, b), bufs=2)
                  for b in range(3)] for a in range(3)]
            for a in range(3):
                for b in range(3):
                    nc.vector.tensor_tensor(g[a][b][:], ay[a][:], bx[b][:],
                                            op=OP.mult)
            return g

        g_next = emit_offsets(0)
        for it in range(11, NR1 // 2):
            conv1_iter(it, pool=ps_f, ptag="pf")
        for blk in range(NBLK):
            r0b = blk * RB
            g = g_next
            if blk + 1 < NBLK:
                g_next = emit_offsets(blk + 1)

            masterT = ps_m.tile([128, RB, 128], F32, tag="masterT")
            masterD = None
            masterP = None
            # open all 4 banks with full-width zero matmuls (sets has_written
            # for the whole region; all later accumulations use start=False)
            for q in range(RB // 4):
                nc.tensor.matmul(
                    masterT[:, q * 4:(q + 1) * 4, :].rearrange(
                        "p a b -> p (a b)"),
                    c_zero[:], xsb0t[:, 0:512], start=True, stop=False)

            def acc_mm(j, lhs_ap, stop=False):
                return nc.tensor.matmul(masterT[:, j, :], lhs_ap, c_id[:],
                                        start=False, stop=stop)

            PDEPTH = int(os.environ.get('K2_PDEPTH', '2'))
            pend = []   # queue of (k, dx, ysl) awaiting term processing
            for ui in range(len(units) + PDEPTH):
                if ui < len(units):
                    k, dx = units[ui]
                    ky, kx = k // 3, k % 3
                    ysl = yp.tile([128, NYR, 128], BF16, tag="ysl")
                    t0 = 0
                    for nch, ch_rows in enumerate(YCH):
                        pt = ps_y.tile([128, ch_rows, 128], F32, tag="ypsum")
                        for tt in range(ch_rows):
                            t = t0 + tt
                            j1 = r0b + t + ky + 1
                            lhs = out1[:, j1, 1 + kx + dx:1 + kx + dx + W]
                            nc.tensor.matmul(pt[:, tt, :], lhs, c_wk[:, k, :],
                                             start=True, stop=True)
                        dst = ysl[:, t0:t0 + ch_rows, :]
                        ev = EVAC[evac_i % len(EVAC)]
                        evac_i += 1
                        if ev == 'A':
                            nc.scalar.copy(dst, pt[:])
                        else:
                            nc.vector.tensor_copy(dst, pt[:])
                        t0 += ch_rows
                    pend.append((k, dx, ysl))

                if ui >= PDEPTH:
                    k, dx, ysl = pend.pop(0)
                    ky, kx = k // 3, k % 3
                    b = dx + 1
                    for dy in (-1, 0, 1):
                        a = dy + 1
                        ti = k * 9 + (dx + 1) * 3 + (dy + 1)
                        me, ae = MULT[ti], ACC[ti]
                        gd = g[a][b]
                        srcv = ysl[:, dy + 1:dy + 1 + RB, :]

                        is_last = (ti == last_e_ti and not has_aD
                                   and not has_aP)

                        def mult_row(dst2, jj, j):
                            if me == 'A':
                                nc.scalar.mul(dst2[:, jj, :],
                                              ysl[:, j + dy + 1, :],
                                              gd[:, j, k:k + 1])
                            else:
                                nc.vector.tensor_scalar(
                                    dst2[:, jj, :], ysl[:, j + dy + 1, :],
                                    gd[:, j, k:k + 1], None, op0=OP.mult)

                        if ae == 'E' and me != 'P':
                            # fine-grained: 4-row tmps, per-row PE accum
                            for q in range(RB // 4):
                                t4 = tmpp.tile([128, 4, 128], BF16, tag="t4")
                                for jj in range(4):
                                    mult_row(t4, jj, q * 4 + jj)
                                for jj in range(4):
                                    acc_mm(q * 4 + jj, t4[:, jj, :],
                                           stop=(is_last and jj == 3))
                            continue

                        # block-granular path
                        if ae == 'D':
                            if masterD is None:
                                masterD = fin.tile([128, RB, 128], BF16,
                                                   tag="masterD", bufs=1)
                                dst, direct = masterD, True
                            else:
                                dst = tmpb.tile([128, RB, 128], BF16, tag="tB")
                                direct = False
                        elif ae == 'P':
                            if masterP is None:
                                masterP = fin.tile([128, RB, 128], BF16,
                                                   tag="masterP", bufs=1)
                                dst, direct = masterP, True
                            else:
                                dst = tmpb.tile([128, RB, 128], BF16, tag="tB")
                                direct = False
                        else:
                            dst = tmpb.tile([128, RB, 128], BF16, tag="tB")
                            direct = False

                        if me == 'P':
                            H2 = RB // int(os.environ.get('K2_PSPLIT', '1'))
                            for h0 in range(0, RB, H2):
                                gb = gd[:, h0:h0 + H2, k:k + 1] \
                                    .broadcast_to([128, H2, 128])
                                nc.gpsimd.tensor_tensor(
                                    dst[:, h0:h0 + H2, :],
                                    ysl[:, dy + 1 + h0:dy + 1 + h0 + H2, :],
                                    gb, op=OP.mult)
                        else:
                            for j in range(RB):
                                mult_row(dst, j, j)

                        if ae == 'E':
                            for j in range(RB):
                                acc_mm(j, dst[:, j, :],
                                       stop=(is_last and j % 4 == 3))
                        elif ae == 'D' and not direct:
                            nc.vector.tensor_tensor(masterD[:], masterD[:],
                                                    dst[:], op=OP.add)
                        elif ae == 'P' and not direct:
                            H2 = RB // int(os.environ.get('K2_PSPLIT', '1'))
                            for h0 in range(0, RB, H2):
                                nc.gpsimd.tensor_tensor(
                                    masterP[:, h0:h0 + H2, :],
                                    masterP[:, h0:h0 + H2, :],
                                    dst[:, h0:h0 + H2, :], op=OP.add)

            # fold pixel-major masters into channel-major PSUM master
            if masterD is not None:
                for j in range(RB):
                    acc_mm(j, masterD[:, j, :],
                           stop=(masterP is None and j % 4 == 3))
            if masterP is not None:
                for j in range(RB):
                    acc_mm(j, masterP[:, j, :], stop=(j % 4 == 3))

            # ---- tail: bn2+relu, conv3, bn3, downsample, final relu (ch-major)
            for q in range(RB // 4):
                out2 = fin.tile([128, 4, 128], BF16, tag="out2")
                nc.scalar.activation(out2[:], masterT[:, q * 4:(q + 1) * 4, :],
                                     AF.Relu, bias=c_bdc2[:, :], scale=1.0)
                pt3 = ps_f.tile([128, 512], F32, tag="pf")
                nc.tensor.matmul(pt3[:], c_w3[:],
                                 out2[:].rearrange("p a b -> p (a b)"),
                                 start=True, stop=True)
                a1 = work.tile([128, 512], F32, tag="a1")
                nc.scalar.activation(a1[:], pt3[:], AF.Relu, bias=c_t3a[:, :],
                                     scale=1.0)
                ptd = ps_f.tile([128, 512], F32, tag="pf")
                px0 = (r0b + 3 + q * 4) * W
                nc.tensor.matmul(ptd[:], c_wds[:, 0, :], xsb0t[:, px0:px0 + 512],
                                 start=True, stop=False)
                nc.tensor.matmul(ptd[:], c_wds[:, 1, :], xsb1t[:, px0:px0 + 512],
                                 start=False, stop=True)
                s1 = work.tile([128, 512], F32, tag="s1")
                nc.vector.scalar_tensor_tensor(s1[:], a1[:], c_s3b[:, :], ptd[:],
                                               op0=OP.mult, op1=OP.add)
                res = fin.tile([128, 512], F32, tag="res")
                nc.scalar.activation(res[:], s1[:], AF.Relu, bias=c_tfin[:, :],
                                     scale=1.0)
                nc.sync.dma_start(
                    out_d[:, (r0b + q * 4) * W:(r0b + q * 4 + 4) * W], res[:])
    return out_d


def _fold(inp):
    f32 = np.float32
    w1full = (inp['w1'] * inp['s1a'][:, None]).astype(f32)
    w1f = np.ascontiguousarray(np.stack(
        [w1full[:, h * 128:(h + 1) * 128].T for h in range(2)], axis=1)).astype(BF)
    woffT = np.ascontiguousarray(np.stack(
        [inp['w_off'][:, :, k // 3, k % 3].T for k in range(KK)], axis=1)).astype(BF)
    s2 = inp['s2']
    wk = np.ascontiguousarray(np.stack(
        [(inp['w_dc'][:, :, k // 3, k % 3] * s2[:, None]).T for k in range(KK)],
        axis=1)).astype(BF)
    bdc2 = (s2 * inp['b_dc'] + inp['t2']).astype(f32)
    w3f = np.ascontiguousarray((inp['w3'] * inp['s3a'][:, None]).T).astype(BF)
    b_dsf = (inp['sd'] * inp['b_ds'] + inp['td']).astype(f32)
    wdsfull = (inp['w_ds'] * inp['sd'][:, None]).astype(f32)
    wdsf = np.ascontiguousarray(np.stack(
        [wdsfull[:, h * 128:(h + 1) * 128].T for h in range(2)], axis=1)).astype(BF)
    col = lambda v: np.ascontiguousarray(np.asarray(v, f32).reshape(-1, 1))
    return {
        'w1f': w1f, 't1a': col(inp['t1a']), 's1b': col(inp['s1b']),
        't1b': col(inp['t1b']), 'woffT': woffT, 'b_off': col(inp['b_off']),
        'wk': wk, 'iden': np.eye(128, dtype=BF),
        'bdc2c': col(bdc2),
        'w3f': w3f, 't3a': col(inp['t3a']), 's3b': col(inp['s3b']),
        'tfin': col(inp['t3b'] + b_dsf), 'wdsf': wdsf,
    }


def _x_slab(x_b, r0):
    sl = np.zeros((256, NR1, W), np.float32)
    lo, hi = r0 - MARG, r0 + ROWS_OUT + MARG
    slo, shi = max(lo, 0), min(hi, H)
    sl[:, slo - lo:shi - lo, :] = x_b[:, slo:shi, :]
    return np.ascontiguousarray(sl.reshape(2, 128, NR1 * W)).astype(BF)


_CACHE = {}


def kernel(**inputs):
    inp = {k: np.asarray(v) for k, v in inputs.items()}
    shared = _fold(inp)
    in_maps = []
    for core in range(N_CORES):
        b, half = core // 2, core % 2
        m = dict(shared)
        m['xs'] = _x_slab(inp['x'][b], half * ROWS_OUT)
        in_maps.append(m)
    if 'nc' not in _CACHE:
        nc = bacc.Bacc()
        _build(nc)
        nc.compile()
        _CACHE['nc'] = nc
    nc = _CACHE['nc']
    res = run_bass_kernel_spmd(nc, in_maps, core_ids=list(range(N_CORES)))
    out = np.zeros((B, PL, H, W), np.float32)
    for core in range(N_CORES):
        b, half = core // 2, core % 2
        r0 = half * ROWS_OUT
        out[b, :, r0:r0 + ROWS_OUT, :] = np.asarray(
            res.results[core]['out'], np.float32).reshape(128, ROWS_OUT, W)
    return out


if __name__ == "__main__":
    pass


# revision 9
# speedup vs baseline: 1.0277x; 1.0092x over previous
"""Trainium2 Bass kernel for nn_DeformBottleneck (DCNv2 bottleneck block), v2.

Same sharding as v1: 8 shards = (batch b, row-half) on 8 cores; each core
computes output rows [r0, r0+64) of one image.

v2 restructure vs v1:
  - Y-field matmuls write fp32 PSUM in small chunks; evacuation split
    across DVE and ACT with a deep chunk ring.
  - The 81 hat-stencil weight terms per row are TS-mults at 4x on DVE
    (per-row [x,ch] tiles, per-partition scalar), with overflow terms on
    ACT (activation scale) and POOL (broadcast tensor_tensor blocks).
  - Accumulation goes to the Tensor engine: weighted tmp tiles transpose-
    accumulate (matmul vs identity) into a channel-major fp32 PSUM master
    [ch, 16, W] per block; leftover terms accumulate into pixel-major
    bf16 masters on DVE/POOL, folded into the PSUM master at block end.
  - Channel-major master kills the conv3 transposes: bn2+relu / conv3 /
    bn3 / downsample / final relu all run channel-major.
"""

import os
import sys
from contextlib import ExitStack

import numpy as np

sys.path.insert(0, "/opt/trn_rl_repo")

import ml_dtypes

import concourse.bass as bass
from concourse import bacc
import concourse.mybir as mybir
import concourse.tile as tile
from concourse.bass_utils import run_bass_kernel_spmd

BF = ml_dtypes.bfloat16
F32 = mybir.dt.float32
BF16 = mybir.dt.bfloat16
I32 = mybir.dt.int32
AF = mybir.ActivationFunctionType
OP = mybir.AluOpType

B, CIN, H, W = 4, 256, 128, 128
PL, KK = 128, 9
PW = 132          # padded out1 slab width
ROWS_OUT = 64     # output rows per core
MARG = 3
NR1 = ROWS_OUT + 2 * MARG
RB = 16           # rows per block
NBLK = ROWS_OUT // RB
NYR = RB + 2
N_CORES = 8

# chunking of the 18-row Y field into PSUM pieces
YCH = (4, 4, 4, 4, 2)


def _sched(weights, n):
    """Interleaved largest-remainder schedule: n picks from weighted set."""
    tot = float(sum(weights.values()))
    acc = {e: 0.0 for e in weights}
    out = []
    for _ in range(n):
        for e in acc:
            acc[e] += weights[e] / tot
        pick = max(acc, key=lambda e: (acc[e], e))
        acc[pick] -= 1.0
        out.append(pick)
    return out


def _parse_w(s, default):
    # "D52A10P19" -> {'D':52,'A':10,'P':19}
    if not s:
        return default
    out = {}
    key = None
    num = ''
    for ch in s:
        if ch.isalpha():
            if key is not None:
                out[key] = int(num)
            key = ch
            num = ''
        else:
            num += ch
    out[key] = int(num)
    return out


def _build(nc):
    MULT_W = _parse_w(os.environ.get('K2_MULT', ''), {'D': 55, 'A': 8, 'P': 18})
    ACC_W = _parse_w(os.environ.get('K2_ACC', ''), {'E': 66, 'D': 8, 'P': 7})
    # per-term engine assignment, term order: (k-major, dx, dy)
    MULT = _sched(MULT_W, 81)
    ACC = _sched(ACC_W, 81)
    if os.environ.get('K2_ACCSORT'):
        ACC = sorted(ACC, key=lambda e: (e == 'E'))
    EVAC = _sched(_parse_w(os.environ.get('K2_EVAC', ''), {'D': 1, 'A': 4}), 60)

    def di(name, shape, dt=F32):
        return nc.dram_tensor(name, shape, dt, kind="ExternalInput")

    xs = di("xs", [2, 128, NR1 * W], BF16)
    w1f = di("w1f", [128, 2, 128], BF16)
    t1a = di("t1a", [128, 1])
    s1b = di("s1b", [128, 1])
    t1b = di("t1b", [128, 1])
    woffT = di("woffT", [128, KK, 27], BF16)
    b_off = di("b_off", [27, 1])
    wk = di("wk", [128, KK, 128], BF16)
    iden = di("iden", [128, 128], BF16)
    bdc2c = di("bdc2c", [128, 1])
    w3f = di("w3f", [128, 128], BF16)
    t3a = di("t3a", [128, 1])
    s3b = di("s3b", [128, 1])
    tfin = di("tfin", [128, 1])
    wdsf = di("wdsf", [128, 2, 128], BF16)
    out_d = nc.dram_tensor("out", [128, ROWS_OUT * W], F32, kind="ExternalOutput")

    with tile.TileContext(nc) as tc, ExitStack() as ctx:
        P = lambda name, bufs=1, **kw: ctx.enter_context(
            tc.tile_pool(name=name, bufs=bufs, **kw))
        consts = P("consts")
        big = P("big")
        wts = P("wts")
        work = P("work", bufs=2)

        c_w1 = consts.tile([128, 2, 128], BF16); nc.sync.dma_start(c_w1[:], w1f[:])
        c_t1a = consts.tile([128, 1], F32); nc.sync.dma_start(c_t1a[:], t1a[:])
        c_s1b = consts.tile([128, 1], F32); nc.sync.dma_start(c_s1b[:], s1b[:])
        c_t1b = consts.tile([128, 1], F32); nc.sync.dma_start(c_t1b[:], t1b[:])
        c_woff = consts.tile([128, KK, 27], BF16); nc.sync.dma_start(c_woff[:], woffT[:])
        c_boff = consts.tile([27, 1], F32); nc.sync.dma_start(c_boff[:], b_off[:])
        c_wk = consts.tile([128, KK, 128], BF16); nc.sync.dma_start(c_wk[:], wk[:])
        c_id = consts.tile([128, 128], BF16); nc.sync.dma_start(c_id[:], iden[:])
        c_zero = consts.tile([128, 128], BF16); nc.vector.memset(c_zero[:], 0.0)
        c_bdc2 = consts.tile([128, 1], F32); nc.sync.dma_start(c_bdc2[:], bdc2c[:])
        c_w3 = consts.tile([128, 128], BF16); nc.sync.dma_start(c_w3[:], w3f[:])
        c_t3a = consts.tile([128, 1], F32); nc.sync.dma_start(c_t3a[:], t3a[:])
        c_s3b = consts.tile([128, 1], F32); nc.sync.dma_start(c_s3b[:], s3b[:])
        c_tfin = consts.tile([128, 1], F32); nc.sync.dma_start(c_tfin[:], tfin[:])
        c_wds = consts.tile([128, 2, 128], BF16); nc.sync.dma_start(c_wds[:], wdsf[:])

        xsb0t = big.tile([128, NR1 * W], BF16)
        xsb1t = big.tile([128, NR1 * W], BF16)
        NQ = NR1 * W // 8
        for q in range(8):
            nc.sync.dma_start(xsb0t[:, q * NQ:(q + 1) * NQ],
                              xs[0, :, q * NQ:(q + 1) * NQ])
            nc.sync.dma_start(xsb1t[:, q * NQ:(q + 1) * NQ],
                              xs[1, :, q * NQ:(q + 1) * NQ])

        out1 = big.tile([128, NR1, PW], BF16)
        nc.gpsimd.memset(out1[:, :, 0:2], 0.0)
        nc.gpsimd.memset(out1[:, :, 130:132], 0.0)

        pre_ctx = tc.tile_pool(name="ps_pre", bufs=2, space="PSUM")
        ps_a = pre_ctx.__enter__()

        # ---- conv1 (1x1 256->128) + BN + relu, twice-relu'd -> out1 slab
        def conv1_iter(it, pool=None, ptag="c1"):
            px0 = it * 2 * W
            pt = (pool or ps_a).tile([128, 2, 128], F32, tag=ptag)
            nc.tensor.matmul(pt[:], c_w1[:, 0, :], xsb0t[:, px0:px0 + 256],
                             start=True, stop=False)
            nc.tensor.matmul(pt[:], c_w1[:, 1, :], xsb1t[:, px0:px0 + 256],
                             start=False, stop=True)
            t = work.tile([128, 2, 128], F32, tag="c1s")
            nc.scalar.activation(t[:], pt[:], AF.Relu, bias=c_t1a[:, :], scale=1.0)
            nc.vector.tensor_scalar(t[:], t[:], c_s1b[:, :], c_t1b[:, :],
                                    op0=OP.mult, op1=OP.add)
            nc.vector.tensor_scalar_max(out1[:, it * 2:it * 2 + 2, 2:130], t[:], 0.0)

        for it in range(11):
            conv1_iter(it)

        pre_ctx.__exit__(None, None, None)
        offp = ctx.enter_context(tc.tile_pool(name="offp", bufs=2))

        ps_y = ctx.enter_context(tc.tile_pool(name="ps_y", bufs=int(os.environ.get("K2_PSYB","3")), space="PSUM"))
        ps_m = ctx.enter_context(tc.tile_pool(name="ps_m", bufs=1, space="PSUM"))
        ps_f = ctx.enter_context(tc.tile_pool(name="ps_f", bufs=1, space="PSUM"))
        yp = ctx.enter_context(tc.tile_pool(name="yp", bufs=int(os.environ.get("K2_YB","5"))))
        tmpp = ctx.enter_context(tc.tile_pool(name="tmpp", bufs=int(os.environ.get("K2_TB","24"))))
        tmpb = ctx.enter_context(tc.tile_pool(name="tmpb", bufs=int(os.environ.get("K2_TBB","4"))))
        fin = ctx.enter_context(tc.tile_pool(name="fin", bufs=2))

        # which accumulate-engines appear (for stop-flag placement)
        has_aD = 'D' in ACC
        has_aP = 'P' in ACC
        last_e_ti = max((ti for ti in range(81) if ACC[ti] == 'E'), default=-1)

        # ---- main loop: per block, stream (k,dx) Y-fields, weight + accum
        units = [(k, dx) for k in range(9) for dx in (-1, 0, 1)]
        evac_i = 0
        def emit_offsets(blk):
            # offsets for block blk: conv, transpose, hat weights g[a][b]
            r0b = blk * RB
            off = offp.tile([27, RB * W], BF16, tag="off")
            for it in range(RB // 4):
                r0 = r0b + it * 4
                pt = ps_f.tile([27, 512], F32, tag="pf")
                for k in range(KK):
                    ky, kx = k // 3, k % 3
                    src = out1[:, r0 + ky + 2:r0 + ky + 6, 1 + kx:1 + kx + W]
                    nc.tensor.matmul(pt[:], c_woff[:, k, :], src,
                                     start=(k == 0), stop=(k == KK - 1))
                nc.scalar.activation(off[:, it * 4 * W:(it + 1) * 4 * W], pt[:],
                                     AF.Identity, bias=c_boff[:, :], scale=1.0)
            offT = offp.tile([128, RB, 28], F32, tag="offT")
            pt = ps_f.tile([128, RB, 28], BF16, tag="pf")
            for j in range(RB):
                nc.tensor.transpose(pt[:, j, 0:27], off[:, j * W:(j + 1) * W],
                                    c_id[0:27, 0:27])
            nc.vector.tensor_copy(offT[:, :, 0:27], pt[:, :, 0:27])

            o1v, o2v, o3v = (offT[:, :, 0:9], offT[:, :, 9:18],
                             offT[:, :, 18:27])
            mask = offp.tile([128, RB, 9], BF16, tag="mask")
            nc.scalar.activation(mask[:], o3v, AF.Sigmoid)
            ay = [offp.tile([128, RB, 9], BF16, name="ayt%d" % i,
                            tag="ayt" + str(i)) for i in range(3)]
            bx = [offp.tile([128, RB, 9], BF16, name="bxt%d" % i,
                            tag="bxt" + str(i)) for i in range(3)]
            tmp = offp.tile([128, RB, 9], BF16, tag="tmp9")
            for (lo, hi, mid, srcv) in ((ay[0], ay[2], ay[1], o1v),
                                        (bx[0], bx[2], bx[1], o2v)):
                nc.vector.tensor_scalar(lo[:], srcv, -1.0, 0.0,
                                        op0=OP.mult, op1=OP.max)
                nc.vector.tensor_scalar_max(hi[:], srcv, 0.0)
                nc.vector.tensor_tensor(tmp[:], lo[:], hi[:], op=OP.add)
                nc.vector.tensor_scalar(mid[:], tmp[:], -1.0, 1.0,
                                        op0=OP.mult, op1=OP.add)
                nc.vector.tensor_scalar_max(mid[:], mid[:], 0.0)
            for i in range(3):
                nc.vector.tensor_tensor(ay[i][:], ay[i][:], mask[:], op=OP.mult)
            g = [[wts.tile([128, RB, 9], F32, name="g%d%d" % (a, b),
                           tag="g%d%d" % (a, b), bufs=2)
                  for b in range(3)] for a in range(3)]
            for a in range(3):
                for b in range(3):
                    nc.vector.tensor_tensor(g[a][b][:], ay[a][:], bx[b][:],
                                            op=OP.mult)
            return g

        g_next = emit_offsets(0)
        for it in range(11, NR1 // 2):
            conv1_iter(it, pool=ps_f, ptag="pf")
        for blk in range(NBLK):
            r0b = blk * RB
            g = g_next
            if blk + 1 < NBLK:
                g_next = emit_offsets(blk + 1)

            masterT = ps_m.tile([128, RB, 128], F32, tag="masterT")
            masterD = None
            masterP = None
            # open all 4 banks with full-width zero matmuls (sets has_written
            # for the whole region; all later accumulations use start=False)
            for q in range(RB // 4):
                nc.tensor.matmul(
                    masterT[:, q * 4:(q + 1) * 4, :].rearrange(
                        "p a b -> p (a b)"),
                    c_zero[:], xsb0t[:, 0:512], start=True, stop=False)

            def acc_mm(j, lhs_ap, stop=False):
                return nc.tensor.matmul(masterT[:, j, :], lhs_ap, c_id[:],
                                        start=False, stop=stop)

            PDEPTH = int(os.environ.get('K2_PDEPTH', '2'))
            pend = []   # queue of (k, dx, ysl) awaiting term processing
            for ui in range(len(units) + PDEPTH):
                if ui < len(units):
                    k, dx = units[ui]
                    ky, kx = k // 3, k % 3
                    ysl = yp.tile([128, NYR, 128], BF16, tag="ysl")
                    t0 = 0
                    for nch, ch_rows in enumerate(YCH):
                        pt = ps_y.tile([128, ch_rows, 128], F32, tag="ypsum")
                        for tt in range(ch_rows):
                            t = t0 + tt
                            j1 = r0b + t + ky + 1
                            lhs = out1[:, j1, 1 + kx + dx:1 + kx + dx + W]
                            nc.tensor.matmul(pt[:, tt, :], lhs, c_wk[:, k, :],
                                             start=True, stop=True)
                        dst = ysl[:, t0:t0 + ch_rows, :]
                        ev = EVAC[evac_i % len(EVAC)]
                        evac_i += 1
                        if ev == 'A':
                            nc.scalar.copy(dst, pt[:])
                        else:
                            nc.vector.tensor_copy(dst, pt[:])
                        t0 += ch_rows
                    pend.append((k, dx, ysl))

                if ui >= PDEPTH:
                    k, dx, ysl = pend.pop(0)
                    ky, kx = k // 3, k % 3
                    b = dx + 1
                    for dy in (-1, 0, 1):
                        a = dy + 1
                        ti = k * 9 + (dx + 1) * 3 + (dy + 1)
                        me, ae = MULT[ti], ACC[ti]
                        gd = g[a][b]
                        srcv = ysl[:, dy + 1:dy + 1 + RB, :]

                        is_last = (ti == last_e_ti and not has_aD
                                   and not has_aP)

                        def mult_row(dst2, jj, j):
                            if me == 'A':
                                nc.scalar.mul(dst2[:, jj, :],
                                              ysl[:, j + dy + 1, :],
                                              gd[:, j, k:k + 1])
                            else:
                                nc.vector.tensor_scalar(
                                    dst2[:, jj, :], ysl[:, j + dy + 1, :],
                                    gd[:, j, k:k + 1], None, op0=OP.mult)

                        if ae == 'E' and me != 'P':
                            # fine-grained: 4-row tmps, per-row PE accum
                            for q in range(RB // 4):
                                t4 = tmpp.tile([128, 4, 128], BF16, tag="t4")
                                for jj in range(4):
                                    mult_row(t4, jj, q * 4 + jj)
                                for jj in range(4):
                                    acc_mm(q * 4 + jj, t4[:, jj, :],
                                           stop=(is_last and jj == 3))
                            continue

                        # block-granular path
                        if ae == 'D':
                            if masterD is None:
                                masterD = fin.tile([128, RB, 128], BF16,
                                                   tag="masterD", bufs=1)
                                dst, direct = masterD, True
                            else:
                                dst = tmpb.tile([128, RB, 128], BF16, tag="tB")
                                direct = False
                        elif ae == 'P':
                            if masterP is None:
                                masterP = fin.tile([128, RB, 128], BF16,
                                                   tag="masterP", bufs=1)
                                dst, direct = masterP, True
                            else:
                                dst = tmpb.tile([128, RB, 128], BF16, tag="tB")
                                direct = False
                        else:
                            dst = tmpb.tile([128, RB, 128], BF16, tag="tB")
                            direct = False

                        if me == 'P':
                            H2 = RB // int(os.environ.get('K2_PSPLIT', '1'))
                            for h0 in range(0, RB, H2):
                                gb = gd[:, h0:h0 + H2, k:k + 1] \
                                    .broadcast_to([128, H2, 128])
                                nc.gpsimd.tensor_tensor(
                                    dst[:, h0:h0 + H2, :],
                                    ysl[:, dy + 1 + h0:dy + 1 + h0 + H2, :],
                                    gb, op=OP.mult)
                        else:
                            for j in range(RB):
                                mult_row(dst, j, j)

                        if ae == 'E':
                            for j in range(RB):
                                acc_mm(j, dst[:, j, :],
                                       stop=(is_last and j % 4 == 3))
                        elif ae == 'D' and not direct:
                            nc.vector.tensor_tensor(masterD[:], masterD[:],
                                                    dst[:], op=OP.add)
                        elif ae == 'P' and not direct:
                            H2 = RB // int(os.environ.get('K2_PSPLIT', '1'))
                            for h0 in range(0, RB, H2):
                                nc.gpsimd.tensor_tensor(
                                    masterP[:, h0:h0 + H2, :],
                                    masterP[:, h0:h0 + H2, :],
                                    dst[:, h0:h0 + H2, :], op=OP.add)

            # fold pixel-major masters into channel-major PSUM master
            if masterD is not None:
                for j in range(RB):
                    acc_mm(j, masterD[:, j, :],
                           stop=(masterP is None and j % 4 == 3))
            if masterP is not None:
                for j in range(RB):
                    acc_mm(j, masterP[:, j, :], stop=(j % 4 == 3))

            # ---- tail: bn2+relu, conv3, bn3, downsample, final relu (ch-major)
            for q in range(RB // 4):
                out2 = fin.tile([128, 4, 128], BF16, tag="out2")
                nc.scalar.activation(out2[:], masterT[:, q * 4:(q + 1) * 4, :],
                                     AF.Relu, bias=c_bdc2[:, :], scale=1.0)
                pt3 = ps_f.tile([128, 512], F32, tag="pf")
                nc.tensor.matmul(pt3[:], c_w3[:],
                                 out2[:].rearrange("p a b -> p (a b)"),
                                 start=True, stop=True)
                a1 = work.tile([128, 512], F32, tag="a1")
                nc.scalar.activation(a1[:], pt3[:], AF.Relu, bias=c_t3a[:, :],
                                     scale=1.0)
                ptd = ps_f.tile([128, 512], F32, tag="pf")
                px0 = (r0b + 3 + q * 4) * W
                nc.tensor.matmul(ptd[:], c_wds[:, 0, :], xsb0t[:, px0:px0 + 512],
                                 start=True, stop=False)
                nc.tensor.matmul(ptd[:], c_wds[:, 1, :], xsb1t[:, px0:px0 + 512],
                                 start=False, stop=True)
                s1 = work.tile([128, 512], F32, tag="s1")
                nc.vector.scalar_tensor_tensor(s1[:], a1[:], c_s3b[:, :], ptd[:],
                                               op0=OP.mult, op1=OP.add)
                res = fin.tile([128, 512], F32, tag="res")
                nc.scalar.activation(res[:], s1[:], AF.Relu, bias=c_tfin[:, :],
                                     scale=1.0)
                nc.sync.dma_start(
                    out_d[:, (r0b + q * 4) * W:(r0b + q * 4 + 4) * W], res[:])
    return out_d


def _fold(inp):
    f32 = np.float32
    w1full = (inp['w1'] * inp['s1a'][:, None]).astype(f32)
    w1f = np.ascontiguousarray(np.stack(
        [w1full[:, h * 128:(h + 1) * 128].T for h in range(2)], axis=1)).astype(BF)
    woffT = np.ascontiguousarray(np.stack(
        [inp['w_off'][:, :, k // 3, k % 3].T for k in range(KK)], axis=1)).astype(BF)
    s2 = inp['s2']
    wk = np.ascontiguousarray(np.stack(
        [(inp['w_dc'][:, :, k // 3, k % 3] * s2[:, None]).T for k in range(KK)],
        axis=1)).astype(BF)
    bdc2 = (s2 * inp['b_dc'] + inp['t2']).astype(f32)
    w3f = np.ascontiguousarray((inp['w3'] * inp['s3a'][:, None]).T).astype(BF)
    b_dsf = (inp['sd'] * inp['b_ds'] + inp['td']).astype(f32)
    wdsfull = (inp['w_ds'] * inp['sd'][:, None]).astype(f32)
    wdsf = np.ascontiguousarray(np.stack(
        [wdsfull[:, h * 128:(h + 1) * 128].T for h in range(2)], axis=1)).astype(BF)
    col = lambda v: np.ascontiguousarray(np.asarray(v, f32).reshape(-1, 1))
    return {
        'w1f': w1f, 't1a': col(inp['t1a']), 's1b': col(inp['s1b']),
        't1b': col(inp['t1b']), 'woffT': woffT, 'b_off': col(inp['b_off']),
        'wk': wk, 'iden': np.eye(128, dtype=BF),
        'bdc2c': col(bdc2),
        'w3f': w3f, 't3a': col(inp['t3a']), 's3b': col(inp['s3b']),
        'tfin': col(inp['t3b'] + b_dsf), 'wdsf': wdsf,
    }


def _x_slab(x_b, r0):
    sl = np.zeros((256, NR1, W), np.float32)
    lo, hi = r0 - MARG, r0 + ROWS_OUT + MARG
    slo, shi = max(lo, 0), min(hi, H)
    sl[:, slo - lo:shi - lo, :] = x_b[:, slo:shi, :]
    return np.ascontiguousarray(sl.reshape(2, 128, NR1 * W)).astype(BF)


_CACHE = {}


def kernel(**inputs):
    inp = {k: np.asarray(v) for k, v in inputs.items()}
    shared = _fold(inp)
    in_maps = []
    for core in range(N_CORES):
        b, half = core // 2, core % 2
        m = dict(shared)
        m['xs'] = _x_slab(inp['x'][b], half * ROWS_OUT)
        in_maps.append(m)
    if 'nc' not in _CACHE:
        nc = bacc.Bacc()
        _build(nc)
        nc.compile()
        _CACHE['nc'] = nc
    nc = _CACHE['nc']
    res = run_bass_kernel_spmd(nc, in_maps, core_ids=list(range(N_CORES)))
    out = np.zeros((B, PL, H, W), np.float32)
    for core in range(N_CORES):
        b, half = core // 2, core % 2
        r0 = half * ROWS_OUT
        out[b, :, r0:r0 + ROWS_OUT, :] = np.asarray(
            res.results[core]['out'], np.float32).reshape(128, ROWS_OUT, W)
    return out


if __name__ == "__main__":
    pass


# revision 10
# speedup vs baseline: 1.0305x; 1.0027x over previous
"""Trainium2 Bass kernel for nn_DeformBottleneck (DCNv2 bottleneck block), v2.

Same sharding as v1: 8 shards = (batch b, row-half) on 8 cores; each core
computes output rows [r0, r0+64) of one image.

v2 restructure vs v1:
  - Y-field matmuls write fp32 PSUM in small chunks; evacuation split
    across DVE and ACT with a deep chunk ring.
  - The 81 hat-stencil weight terms per row are TS-mults at 4x on DVE
    (per-row [x,ch] tiles, per-partition scalar), with overflow terms on
    ACT (activation scale) and POOL (broadcast tensor_tensor blocks).
  - Accumulation goes to the Tensor engine: weighted tmp tiles transpose-
    accumulate (matmul vs identity) into a channel-major fp32 PSUM master
    [ch, 16, W] per block; leftover terms accumulate into pixel-major
    bf16 masters on DVE/POOL, folded into the PSUM master at block end.
  - Channel-major master kills the conv3 transposes: bn2+relu / conv3 /
    bn3 / downsample / final relu all run channel-major.
"""

import os
import sys
from contextlib import ExitStack

import numpy as np

sys.path.insert(0, "/opt/trn_rl_repo")

import ml_dtypes

import concourse.bass as bass
from concourse import bacc
import concourse.mybir as mybir
import concourse.tile as tile
from concourse.bass_utils import run_bass_kernel_spmd

BF = ml_dtypes.bfloat16
F32 = mybir.dt.float32
BF16 = mybir.dt.bfloat16
I32 = mybir.dt.int32
AF = mybir.ActivationFunctionType
OP = mybir.AluOpType

B, CIN, H, W = 4, 256, 128, 128
PL, KK = 128, 9
PW = 132          # padded out1 slab width
ROWS_OUT = 64     # output rows per core
MARG = 3
NR1 = ROWS_OUT + 2 * MARG
RB = 16           # rows per block
NBLK = ROWS_OUT // RB
NYR = RB + 2
N_CORES = 8

# chunking of the 18-row Y field into PSUM pieces
YCH = (4, 4, 4, 4, 2)


def _sched(weights, n):
    """Interleaved largest-remainder schedule: n picks from weighted set."""
    tot = float(sum(weights.values()))
    acc = {e: 0.0 for e in weights}
    out = []
    for _ in range(n):
        for e in acc:
            acc[e] += weights[e] / tot
        pick = max(acc, key=lambda e: (acc[e], e))
        acc[pick] -= 1.0
        out.append(pick)
    return out


def _parse_w(s, default):
    # "D52A10P19" -> {'D':52,'A':10,'P':19}
    if not s:
        return default
    out = {}
    key = None
    num = ''
    for ch in s:
        if ch.isalpha():
            if key is not None:
                out[key] = int(num)
            key = ch
            num = ''
        else:
            num += ch
    out[key] = int(num)
    return out


def _build(nc):
    MULT_W = _parse_w(os.environ.get('K2_MULT', ''), {'D': 55, 'A': 8, 'P': 18})
    ACC_W = _parse_w(os.environ.get('K2_ACC', ''), {'E': 66, 'D': 8, 'P': 7})
    # per-term engine assignment, term order: (k-major, dx, dy)
    MULT = _sched(MULT_W, 81)
    ACC = _sched(ACC_W, 81)
    if os.environ.get('K2_ACCSORT'):
        ACC = sorted(ACC, key=lambda e: (e == 'E'))
    EVAC = _sched(_parse_w(os.environ.get('K2_EVAC', ''), {'D': 1, 'A': 6}), 60)

    def di(name, shape, dt=F32):
        return nc.dram_tensor(name, shape, dt, kind="ExternalInput")

    xs = di("xs", [2, 128, NR1 * W], BF16)
    w1f = di("w1f", [128, 2, 128], BF16)
    t1a = di("t1a", [128, 1])
    s1b = di("s1b", [128, 1])
    t1b = di("t1b", [128, 1])
    woffT = di("woffT", [128, KK, 27], BF16)
    b_off = di("b_off", [27, 1])
    wk = di("wk", [128, KK, 128], BF16)
    iden = di("iden", [128, 128], BF16)
    bdc2c = di("bdc2c", [128, 1])
    w3f = di("w3f", [128, 128], BF16)
    t3a = di("t3a", [128, 1])
    s3b = di("s3b", [128, 1])
    tfin = di("tfin", [128, 1])
    wdsf = di("wdsf", [128, 2, 128], BF16)
    out_d = nc.dram_tensor("out", [128, ROWS_OUT * W], F32, kind="ExternalOutput")

    with tile.TileContext(nc) as tc, ExitStack() as ctx:
        P = lambda name, bufs=1, **kw: ctx.enter_context(
            tc.tile_pool(name=name, bufs=bufs, **kw))
        consts = P("consts")
        big = P("big")
        wts = P("wts")
        work = P("work", bufs=2)

        c_w1 = consts.tile([128, 2, 128], BF16); nc.sync.dma_start(c_w1[:], w1f[:])
        c_t1a = consts.tile([128, 1], F32); nc.sync.dma_start(c_t1a[:], t1a[:])
        c_s1b = consts.tile([128, 1], F32); nc.sync.dma_start(c_s1b[:], s1b[:])
        c_t1b = consts.tile([128, 1], F32); nc.sync.dma_start(c_t1b[:], t1b[:])
        c_woff = consts.tile([128, KK, 27], BF16); nc.sync.dma_start(c_woff[:], woffT[:])
        c_boff = consts.tile([27, 1], F32); nc.sync.dma_start(c_boff[:], b_off[:])
        c_wk = consts.tile([128, KK, 128], BF16); nc.sync.dma_start(c_wk[:], wk[:])
        c_id = consts.tile([128, 128], BF16); nc.sync.dma_start(c_id[:], iden[:])
        c_zero = consts.tile([128, 128], BF16); nc.vector.memset(c_zero[:], 0.0)
        c_bdc2 = consts.tile([128, 1], F32); nc.sync.dma_start(c_bdc2[:], bdc2c[:])
        c_w3 = consts.tile([128, 128], BF16); nc.sync.dma_start(c_w3[:], w3f[:])
        c_t3a = consts.tile([128, 1], F32); nc.sync.dma_start(c_t3a[:], t3a[:])
        c_s3b = consts.tile([128, 1], F32); nc.sync.dma_start(c_s3b[:], s3b[:])
        c_tfin = consts.tile([128, 1], F32); nc.sync.dma_start(c_tfin[:], tfin[:])
        c_wds = consts.tile([128, 2, 128], BF16); nc.sync.dma_start(c_wds[:], wdsf[:])

        xsb0t = big.tile([128, NR1 * W], BF16)
        xsb1t = big.tile([128, NR1 * W], BF16)
        NQ = NR1 * W // 8
        for q in range(8):
            nc.sync.dma_start(xsb0t[:, q * NQ:(q + 1) * NQ],
                              xs[0, :, q * NQ:(q + 1) * NQ])
            nc.sync.dma_start(xsb1t[:, q * NQ:(q + 1) * NQ],
                              xs[1, :, q * NQ:(q + 1) * NQ])

        out1 = big.tile([128, NR1, PW], BF16)
        nc.gpsimd.memset(out1[:, :, 0:2], 0.0)
        nc.gpsimd.memset(out1[:, :, 130:132], 0.0)

        pre_ctx = tc.tile_pool(name="ps_pre", bufs=2, space="PSUM")
        ps_a = pre_ctx.__enter__()

        # ---- conv1 (1x1 256->128) + BN + relu, twice-relu'd -> out1 slab
        def conv1_iter(it, pool=None, ptag="c1"):
            px0 = it * 2 * W
            pt = (pool or ps_a).tile([128, 2, 128], F32, tag=ptag)
            nc.tensor.matmul(pt[:], c_w1[:, 0, :], xsb0t[:, px0:px0 + 256],
                             start=True, stop=False)
            nc.tensor.matmul(pt[:], c_w1[:, 1, :], xsb1t[:, px0:px0 + 256],
                             start=False, stop=True)
            t = work.tile([128, 2, 128], F32, tag="c1s")
            nc.scalar.activation(t[:], pt[:], AF.Relu, bias=c_t1a[:, :], scale=1.0)
            nc.vector.tensor_scalar(t[:], t[:], c_s1b[:, :], c_t1b[:, :],
                                    op0=OP.mult, op1=OP.add)
            nc.vector.tensor_scalar_max(out1[:, it * 2:it * 2 + 2, 2:130], t[:], 0.0)

        for it in range(11):
            conv1_iter(it)

        pre_ctx.__exit__(None, None, None)
        offp = ctx.enter_context(tc.tile_pool(name="offp", bufs=2))

        ps_y = ctx.enter_context(tc.tile_pool(name="ps_y", bufs=int(os.environ.get("K2_PSYB","3")), space="PSUM"))
        ps_m = ctx.enter_context(tc.tile_pool(name="ps_m", bufs=1, space="PSUM"))
        ps_f = ctx.enter_context(tc.tile_pool(name="ps_f", bufs=1, space="PSUM"))
        yp = ctx.enter_context(tc.tile_pool(name="yp", bufs=int(os.environ.get("K2_YB","5"))))
        tmpp = ctx.enter_context(tc.tile_pool(name="tmpp", bufs=int(os.environ.get("K2_TB","24"))))
        tmpb = ctx.enter_context(tc.tile_pool(name="tmpb", bufs=int(os.environ.get("K2_TBB","4"))))
        fin = ctx.enter_context(tc.tile_pool(name="fin", bufs=2))

        # which accumulate-engines appear (for stop-flag placement)
        has_aD = 'D' in ACC
        has_aP = 'P' in ACC
        last_e_ti = max((ti for ti in range(81) if ACC[ti] == 'E'), default=-1)

        # ---- main loop: per block, stream (k,dx) Y-fields, weight + accum
        units = [(k, dx) for k in range(9) for dx in (-1, 0, 1)]
        evac_i = 0
        def emit_offsets(blk):
            # offsets for block blk: conv, transpose, hat weights g[a][b]
            r0b = blk * RB
            off = offp.tile([27, RB * W], BF16, tag="off")
            for it in range(RB // 4):
                r0 = r0b + it * 4
                pt = ps_f.tile([27, 512], F32, tag="pf")
                for k in range(KK):
                    ky, kx = k // 3, k % 3
                    src = out1[:, r0 + ky + 2:r0 + ky + 6, 1 + kx:1 + kx + W]
                    nc.tensor.matmul(pt[:], c_woff[:, k, :], src,
                                     start=(k == 0), stop=(k == KK - 1))
                nc.scalar.activation(off[:, it * 4 * W:(it + 1) * 4 * W], pt[:],
                                     AF.Identity, bias=c_boff[:, :], scale=1.0)
            offT = offp.tile([128, RB, 28], F32, tag="offT")
            pt = ps_f.tile([128, RB, 28], BF16, tag="pf")
            for j in range(RB):
                nc.tensor.transpose(pt[:, j, 0:27], off[:, j * W:(j + 1) * W],
                                    c_id[0:27, 0:27])
            nc.vector.tensor_copy(offT[:, :, 0:27], pt[:, :, 0:27])

            o1v, o2v, o3v = (offT[:, :, 0:9], offT[:, :, 9:18],
                             offT[:, :, 18:27])
            mask = offp.tile([128, RB, 9], BF16, tag="mask")
            nc.scalar.activation(mask[:], o3v, AF.Sigmoid)
            ay = [offp.tile([128, RB, 9], BF16, name="ayt%d" % i,
                            tag="ayt" + str(i)) for i in range(3)]
            bx = [offp.tile([128, RB, 9], BF16, name="bxt%d" % i,
                            tag="bxt" + str(i)) for i in range(3)]
            tmp = offp.tile([128, RB, 9], BF16, tag="tmp9")
            for (lo, hi, mid, srcv) in ((ay[0], ay[2], ay[1], o1v),
                                        (bx[0], bx[2], bx[1], o2v)):
                nc.vector.tensor_scalar(lo[:], srcv, -1.0, 0.0,
                                        op0=OP.mult, op1=OP.max)
                nc.vector.tensor_scalar_max(hi[:], srcv, 0.0)
                nc.vector.tensor_tensor(tmp[:], lo[:], hi[:], op=OP.add)
                nc.vector.tensor_scalar(mid[:], tmp[:], -1.0, 1.0,
                                        op0=OP.mult, op1=OP.add)
                nc.vector.tensor_scalar_max(mid[:], mid[:], 0.0)
            for i in range(3):
                nc.vector.tensor_tensor(ay[i][:], ay[i][:], mask[:], op=OP.mult)
            g = [[wts.tile([128, RB, 9], F32, name="g%d%d" % (a, b),
                           tag="g%d%d" % (a, b), bufs=2)
                  for b in range(3)] for a in range(3)]
            for a in range(3):
                for b in range(3):
                    if os.environ.get('K2_GPOOL', '0') == '1':
                        nc.gpsimd.tensor_tensor(g[a][b][:], ay[a][:], bx[b][:],
                                                op=OP.mult)
                    else:
                        nc.vector.tensor_tensor(g[a][b][:], ay[a][:], bx[b][:],
                                                op=OP.mult)
            return g

        g_next = emit_offsets(0)
        for it in range(11, NR1 // 2):
            conv1_iter(it, pool=ps_f, ptag="pf")
        for blk in range(NBLK):
            r0b = blk * RB
            g = g_next
            if blk + 1 < NBLK:
                g_next = emit_offsets(blk + 1)

            masterT = ps_m.tile([128, RB, 128], F32, tag="masterT")
            masterD = None
            masterP = None
            # open all 4 banks with full-width zero matmuls (sets has_written
            # for the whole region; all later accumulations use start=False)
            for q in range(RB // 4):
                nc.tensor.matmul(
                    masterT[:, q * 4:(q + 1) * 4, :].rearrange(
                        "p a b -> p (a b)"),
                    c_zero[:], xsb0t[:, 0:512], start=True, stop=False)

            def acc_mm(j, lhs_ap, stop=False):
                return nc.tensor.matmul(masterT[:, j, :], lhs_ap, c_id[:],
                                        start=False, stop=stop)

            PDEPTH = int(os.environ.get('K2_PDEPTH', '2'))
            pend = []   # queue of (k, dx, ysl) awaiting term processing
            for ui in range(len(units) + PDEPTH):
                if ui < len(units):
                    k, dx = units[ui]
                    ky, kx = k // 3, k % 3
                    ysl = yp.tile([128, NYR, 128], BF16, tag="ysl")
                    t0 = 0
                    for nch, ch_rows in enumerate(YCH):
                        pt = ps_y.tile([128, ch_rows, 128], F32, tag="ypsum")
                        for tt in range(ch_rows):
                            t = t0 + tt
                            j1 = r0b + t + ky + 1
                            lhs = out1[:, j1, 1 + kx + dx:1 + kx + dx + W]
                            nc.tensor.matmul(pt[:, tt, :], lhs, c_wk[:, k, :],
                                             start=True, stop=True)
                        dst = ysl[:, t0:t0 + ch_rows, :]
                        ev = EVAC[evac_i % len(EVAC)]
                        evac_i += 1
                        if ev == 'A':
                            nc.scalar.copy(dst, pt[:])
                        else:
                            nc.vector.tensor_copy(dst, pt[:])
                        t0 += ch_rows
                    pend.append((k, dx, ysl))

                if ui >= PDEPTH:
                    k, dx, ysl = pend.pop(0)
                    ky, kx = k // 3, k % 3
                    b = dx + 1
                    for dy in (-1, 0, 1):
                        a = dy + 1
                        ti = k * 9 + (dx + 1) * 3 + (dy + 1)
                        me, ae = MULT[ti], ACC[ti]
                        gd = g[a][b]
                        srcv = ysl[:, dy + 1:dy + 1 + RB, :]

                        is_last = (ti == last_e_ti and not has_aD
                                   and not has_aP)

                        def mult_row(dst2, jj, j):
                            if me == 'A':
                                nc.scalar.mul(dst2[:, jj, :],
                                              ysl[:, j + dy + 1, :],
                                              gd[:, j, k:k + 1])
                            else:
                                nc.vector.tensor_scalar(
                                    dst2[:, jj, :], ysl[:, j + dy + 1, :],
                                    gd[:, j, k:k + 1], None, op0=OP.mult)

                        if ae == 'E' and me != 'P':
                            # fine-grained: 4-row tmps, per-row PE accum
                            for q in range(RB // 4):
                                t4 = tmpp.tile([128, 4, 128], BF16, tag="t4")
                                for jj in range(4):
                                    mult_row(t4, jj, q * 4 + jj)
                                for jj in range(4):
                                    acc_mm(q * 4 + jj, t4[:, jj, :],
                                           stop=(is_last and jj == 3))
                            continue

                        # block-granular path
                        if ae == 'D':
                            if masterD is None:
                                masterD = fin.tile([128, RB, 128], BF16,
                                                   tag="masterD", bufs=1)
                                dst, direct = masterD, True
                            else:
                                dst = tmpb.tile([128, RB, 128], BF16, tag="tB")
                                direct = False
                        elif ae == 'P':
                            if masterP is None:
                                masterP = fin.tile([128, RB, 128], BF16,
                                                   tag="masterP", bufs=1)
                                dst, direct = masterP, True
                            else:
                                dst = tmpb.tile([128, RB, 128], BF16, tag="tB")
                                direct = False
                        else:
                            dst = tmpb.tile([128, RB, 128], BF16, tag="tB")
                            direct = False

                        if me == 'P':
                            H2 = RB // int(os.environ.get('K2_PSPLIT', '1'))
                            for h0 in range(0, RB, H2):
                                gb = gd[:, h0:h0 + H2, k:k + 1] \
                                    .broadcast_to([128, H2, 128])
                                nc.gpsimd.tensor_tensor(
                                    dst[:, h0:h0 + H2, :],
                                    ysl[:, dy + 1 + h0:dy + 1 + h0 + H2, :],
                                    gb, op=OP.mult)
                        else:
                            for j in range(RB):
                                mult_row(dst, j, j)

                        if ae == 'E':
                            for j in range(RB):
                                acc_mm(j, dst[:, j, :],
                                       stop=(is_last and j % 4 == 3))
                        elif ae == 'D' and not direct:
                            nc.vector.tensor_tensor(masterD[:], masterD[:],
                                                    dst[:], op=OP.add)
                        elif ae == 'P' and not direct:
                            H2 = RB // int(os.environ.get('K2_PSPLIT', '1'))
                            for h0 in range(0, RB, H2):
                                nc.gpsimd.tensor_tensor(
                                    masterP[:, h0:h0 + H2, :],
                                    masterP[:, h0:h0 + H2, :],
                                    dst[:, h0:h0 + H2, :], op=OP.add)

            # fold pixel-major masters into channel-major PSUM master
            if masterD is not None:
                for j in range(RB):
                    acc_mm(j, masterD[:, j, :],
                           stop=(masterP is None and j % 4 == 3))
            if masterP is not None:
                for j in range(RB):
                    acc_mm(j, masterP[:, j, :], stop=(j % 4 == 3))

            # ---- tail: bn2+relu, conv3, bn3, downsample, final relu (ch-major)
            for q in range(RB // 4):
                out2 = fin.tile([128, 4, 128], BF16, tag="out2")
                nc.scalar.activation(out2[:], masterT[:, q * 4:(q + 1) * 4, :],
                                     AF.Relu, bias=c_bdc2[:, :], scale=1.0)
                pt3 = ps_f.tile([128, 512], F32, tag="pf")
                nc.tensor.matmul(pt3[:], c_w3[:],
                                 out2[:].rearrange("p a b -> p (a b)"),
                                 start=True, stop=True)
                a1 = work.tile([128, 512], F32, tag="a1")
                nc.scalar.activation(a1[:], pt3[:], AF.Relu, bias=c_t3a[:, :],
                                     scale=1.0)
                ptd = ps_f.tile([128, 512], F32, tag="pf")
                px0 = (r0b + 3 + q * 4) * W
                nc.tensor.matmul(ptd[:], c_wds[:, 0, :], xsb0t[:, px0:px0 + 512],
                                 start=True, stop=False)
                nc.tensor.matmul(ptd[:], c_wds[:, 1, :], xsb1t[:, px0:px0 + 512],
                                 start=False, stop=True)
                s1 = work.tile([128, 512], F32, tag="s1")
                nc.vector.scalar_tensor_tensor(s1[:], a1[:], c_s3b[:, :], ptd[:],
                                               op0=OP.mult, op1=OP.add)
                res = fin.tile([128, 512], F32, tag="res")
                nc.scalar.activation(res[:], s1[:], AF.Relu, bias=c_tfin[:, :],
                                     scale=1.0)
                nc.sync.dma_start(
                    out_d[:, (r0b + q * 4) * W:(r0b + q * 4 + 4) * W], res[:])
    return out_d


def _fold(inp):
    f32 = np.float32
    w1full = (inp['w1'] * inp['s1a'][:, None]).astype(f32)
    w1f = np.ascontiguousarray(np.stack(
        [w1full[:, h * 128:(h + 1) * 128].T for h in range(2)], axis=1)).astype(BF)
    woffT = np.ascontiguousarray(np.stack(
        [inp['w_off'][:, :, k // 3, k % 3].T for k in range(KK)], axis=1)).astype(BF)
    s2 = inp['s2']
    wk = np.ascontiguousarray(np.stack(
        [(inp['w_dc'][:, :, k // 3, k % 3] * s2[:, None]).T for k in range(KK)],
        axis=1)).astype(BF)
    bdc2 = (s2 * inp['b_dc'] + inp['t2']).astype(f32)
    w3f = np.ascontiguousarray((inp['w3'] * inp['s3a'][:, None]).T).astype(BF)
    b_dsf = (inp['sd'] * inp['b_ds'] + inp['td']).astype(f32)
    wdsfull = (inp['w_ds'] * inp['sd'][:, None]).astype(f32)
    wdsf = np.ascontiguousarray(np.stack(
        [wdsfull[:, h * 128:(h + 1) * 128].T for h in range(2)], axis=1)).astype(BF)
    col = lambda v: np.ascontiguousarray(np.asarray(v, f32).reshape(-1, 1))
    return {
        'w1f': w1f, 't1a': col(inp['t1a']), 's1b': col(inp['s1b']),
        't1b': col(inp['t1b']), 'woffT': woffT, 'b_off': col(inp['b_off']),
        'wk': wk, 'iden': np.eye(128, dtype=BF),
        'bdc2c': col(bdc2),
        'w3f': w3f, 't3a': col(inp['t3a']), 's3b': col(inp['s3b']),
        'tfin': col(inp['t3b'] + b_dsf), 'wdsf': wdsf,
    }


def _x_slab(x_b, r0):
    sl = np.zeros((256, NR1, W), np.float32)
    lo, hi = r0 - MARG, r0 + ROWS_OUT + MARG
    slo, shi = max(lo, 0), min(hi, H)
    sl[:, slo - lo:shi - lo, :] = x_b[:, slo:shi, :]
    return np.ascontiguousarray(sl.reshape(2, 128, NR1 * W)).astype(BF)


_CACHE = {}


def kernel(**inputs):
    inp = {k: np.asarray(v) for k, v in inputs.items()}
    shared = _fold(inp)
    in_maps = []
    for core in range(N_CORES):
        b, half = core // 2, core % 2
        m = dict(shared)
        m['xs'] = _x_slab(inp['x'][b], half * ROWS_OUT)
        in_maps.append(m)
    if 'nc' not in _CACHE:
        nc = bacc.Bacc()
        _build(nc)
        nc.compile()
        _CACHE['nc'] = nc
    nc = _CACHE['nc']
    res = run_bass_kernel_spmd(nc, in_maps, core_ids=list(range(N_CORES)))
    out = np.zeros((B, PL, H, W), np.float32)
    for core in range(N_CORES):
        b, half = core // 2, core % 2
        r0 = half * ROWS_OUT
        out[b, :, r0:r0 + ROWS_OUT, :] = np.asarray(
            res.results[core]['out'], np.float32).reshape(128, ROWS_OUT, W)
    return out


if __name__ == "__main__":
    pass
